# revision 39
# baseline (speedup 1.0000x reference)
"""Trainium2 Bass kernel: 32-head GQA attention prefill (Llama-style),
tensor-parallel over heads across 8 NeuronCores.

v3: block-pipelined schedule. Per s-block j the attention work (scores ->
exp on ACT -> AV) is interleaved on the PE with "filler" units taken from
the NEXT block's projections and the PREVIOUS block's output projection,
so the ACT engine's ~113us of exp work hides under PE matmul work instead
of serializing phase C (v2 lost ~35us there). Other v3 changes:
  - wq and wo fully resident in SBUF (no per-block wq reloads: -12MB HBM)
  - block-0 x loads split into halves across the sync+scalar hw DGE
    queues, first projection groups run at 256 cols -> lead-in ~9us vs 22
  - bf16 output partials staged into [128,4096] rows, 16 big stores on the
    Pool engine's software DGE (keeps sync/scalar queues for loads)
  - rope / mask / evacuation work split across DVE and the idle Pool
    engine

Math (per core m) unchanged from v2:
  local Q heads H = 4m..4m+3, local KV head = m.
  qT_h [hd,s] = wqT_h chunks @ xT (bf16, fp32 psum), RoPE fused on psum
  (head-dim de-interleaved host-side so rope pairs are partition halves).
  scoresT [sk,sq] = kT-chunk.T @ qT -> exp(./sqrt(128)) -> eT bf16,
  causal via chunk skipping + 0/1 mask-mul on diagonal chunks.
  out_aug [sq,129] = eT-chunks.T @ [v | 1]; attn = out_aug[:, :128]/rowsum,
  PE-transposed to attnT; partial_out = attnT-chunks.T @ woT.
  Host sums the 8 per-core bf16 partials (the "all-reduce after wo").
"""

import sys

sys.path.insert(0, "/opt/trn_rl_repo")

import math
from collections import deque

import ml_dtypes
import numpy as np

# bass_utils' trace branch imports antenv.axon_hooks, which some images
# lack; install a null shim so a stray BASS_TRACE env can't crash us.
try:
    import antenv.axon_hooks  # noqa: F401
except ImportError:
    import types as _types

    _hm = _types.ModuleType("antenv.axon_hooks")
    _hm._hook = None
    _hm.set_axon_ntff_profile_hook = lambda h: setattr(_hm, "_hook", h)
    _hm.get_axon_ntff_profile_hook = lambda: _hm._hook
    sys.modules["antenv.axon_hooks"] = _hm
    try:
        import antenv as _antenv

        _antenv.axon_hooks = _hm
    except ImportError:
        pass

DIM = 4096
NCORES = 8
HQ = 4  # local q heads per core
HD = 128


LAST_EXEC_NS = None
LAST_RESULT = None

_HEAD_PERM = np.concatenate([np.arange(0, HD, 2), np.arange(1, HD, 2)])


def _build_causal(S):
    """Fused causal kernel (v3). Returns nc."""
    import concourse.mybir as mybir
    import concourse.tile as tile
    from concourse import bacc

    dt = mybir.dt
    BF, F32 = dt.bfloat16, dt.float32
    NB = S // 512  # s blocks
    NT = S // 128  # s tiles == sk chunks
    ND = DIM // 128  # d chunks
    DG = ND // 4  # d-chunks per x part
    SC = 1.0 / math.sqrt(HD)

    nc = bacc.Bacc(None, target_bir_lowering=False)
    # x is block-major so each part-load is 8KB-contiguous per partition
    # (the [128, ND, S] layout produced 512B DMA packets that throttled the
    # early loads to ~100GB/s and starved proj(0))
    xT = nc.declare_dram_parameter("xT", [128, NB, ND, 512], BF, isOutput=False)
    # block 0 duplicated half-major so its 256-col half loads stay 4KB
    # contiguous per partition (halving the block-major layout would drop
    # DMA packets back to 512B)
    x0T = nc.declare_dram_parameter("x0T", [128, 2, ND, 256], BF, isOutput=False)
    wqT = nc.declare_dram_parameter("wqT", [HQ, 128, ND, 128], BF, isOutput=False)
    wkT = nc.declare_dram_parameter("wkT", [128, ND, 128], BF, isOutput=False)
    wvT = nc.declare_dram_parameter("wvT", [128, ND, 128], BF, isOutput=False)
    woT = nc.declare_dram_parameter("woT", [128, HQ, DIM], BF, isOutput=False)
    cosT = nc.declare_dram_parameter("cosT", [64, S], F32, isOutput=False)
    sinT = nc.declare_dram_parameter("sinT", [64, S], F32, isOutput=False)
    identD = nc.declare_dram_parameter("identD", [128, 128], BF, isOutput=False)
    dmaskD = nc.declare_dram_parameter("dmaskD", [128, 4, 512], BF, isOutput=False)
    outD = nc.declare_dram_parameter("out", [S, DIM], BF, isOutput=True)

    with tile.TileContext(nc) as tc:
        with (
            tc.tile_pool(name="persist", bufs=1) as pp,
            tc.tile_pool(name="xb", bufs=6) as xp,
            tc.tile_pool(name="blk", bufs=2) as bp,
            tc.tile_pool(name="et", bufs=NT + 1) as ep,
            tc.tile_pool(name="small", bufs=1) as sp,
            tc.tile_pool(name="stage", bufs=2) as op,
            tc.tile_pool(name="ps_proj", bufs=3, space="PSUM") as psP,
            tc.tile_pool(name="ps_scores", bufs=2, space="PSUM") as psS,
            tc.tile_pool(name="ps_oaug", bufs=2, space="PSUM") as psO,
            tc.tile_pool(name="ps_trans", bufs=1, space="PSUM") as psT,
        ):
            # ---- persistent tiles ----
            ident = pp.tile([128, 128], BF, tag="ident", name="ident")
            dmask = pp.tile([128, 4, 512], BF, tag="dmask", name="dmask")
            wk_t = pp.tile([128, ND, 128], BF, tag="wk", name="wk")
            wv_t = pp.tile([128, ND, 128], BF, tag="wv", name="wv")
            wq_t = [
                pp.tile([128, ND, 128], BF, tag=f"wq{h}", name=f"wq{h}")
                for h in range(HQ)
            ]
            wo_t = pp.tile([128, HQ, DIM], BF, tag="wo", name="wo")
            kT = pp.tile([128, S], BF, tag="kT", name="kT")
            vaug = pp.tile([128, NT, 129], BF, tag="vaug", name="vaug")
            nc.gpsimd.memset(vaug[:, :, 128:129], 1.0)

            # rotating per-block tiles
            def blk_tile(shape, dtp, tag):
                return bp.tile(shape, dtp, tag=tag, name=tag)

            # ---- initial DMA schedule ----
            # scalar (ACT hw DGE): small consts, half of x block0, wv, wo,
            #   cos/sin block1, x block1 p0/p1 (ACT is exp-busy later).
            # sync (SP hw DGE): other half of x block0, wq0-3, x b1 p2/p3;
            #   later blocks' x/cos/sin emitted at block boundaries.
            # gpsimd (Pool sw DGE): output stores only.
            nc.scalar.dma_start(out=wk_t[:], in_=wkT[:])
            nc.scalar.dma_start(out=ident[:], in_=identD[:])
            nc.scalar.dma_start(out=dmask[:], in_=dmaskD[:])

            cos_b = {}
            sin_b = {}

            def load_cossin(j, eng):
                jsl = slice(j * 512, (j + 1) * 512)
                cos_b[j] = blk_tile([64, 512], F32, "cosb")
                sin_b[j] = blk_tile([64, 512], F32, "sinb")
                eng.dma_start(out=cos_b[j][:], in_=cosT[:, jsl])
                eng.dma_start(out=sin_b[j][:], in_=sinT[:, jsl])

            load_cossin(0, nc.scalar)

            x_parts = {}  # (j, g) -> tile [128, DG, 512]

            def alloc_x(j):
                for g in range(4):
                    x_parts[(j, g)] = xp.tile(
                        [128, DG, 512], BF, tag="xb", name="xb"
                    )

            def load_x_part(j, g, eng, half=None):
                t = x_parts[(j, g)]
                if half is None:
                    eng.dma_start(out=t[:], in_=xT[:, j, g * DG : (g + 1) * DG, :])
                else:
                    eng.dma_start(
                        out=t[:, :, half * 256 : half * 256 + 256],
                        in_=x0T[:, half, g * DG : (g + 1) * DG, :],
                    )

            # block 0: halves split across the two hw queues so the first
            # 256-col projection groups can start ~9us in. Each queue is
            # ordered by first-use time: x halves, then wq (Q groups),
            # then block-1 x / cos/sin, then wo (first needed at D(0)).
            alloc_x(0)
            if NB > 1:
                alloc_x(1)
            # arrival order matches group consumption order:
            # K(a) V(a) K(b) V(b) inline, then Q0..Q3 as attn(0) filler
            # (sw-DGE loads on Pool would block the rope combines behind
            # descriptor generation — keep everything on the hw queues)
            for g in (0, 1):
                load_x_part(0, g, nc.sync, half=0)
            for g in (2, 3):
                load_x_part(0, g, nc.scalar, half=0)
            nc.scalar.dma_start(out=wv_t[:], in_=wvT[:])
            nc.sync.dma_start(out=wq_t[0][:], in_=wqT[0])
            for g in (0, 1):
                load_x_part(0, g, nc.sync, half=1)
            for g in (2, 3):
                load_x_part(0, g, nc.scalar, half=1)
            nc.sync.dma_start(out=wq_t[1][:], in_=wqT[1])
            nc.scalar.dma_start(out=wq_t[2][:], in_=wqT[2])
            nc.scalar.dma_start(out=wq_t[3][:], in_=wqT[3])
            if NB > 1:
                load_cossin(1, nc.scalar)
                load_x_part(1, 2, nc.sync)
                load_x_part(1, 3, nc.sync)
                load_x_part(1, 0, nc.sync)
                load_x_part(1, 1, nc.scalar)
            nc.scalar.dma_start(out=wo_t[:], in_=woT[:])

            def xbd(j, d):
                return x_parts[(j, d // DG)][:, d % DG, :]

            # ---- rope (psum [128,512] f32 -> dst bf16) ----
            # Pool can't read PSUM: the four products (psum reads) go on DVE,
            # the two SBUF-only combine steps on Pool.
            def rope(ps, dst, j, csl=slice(0, 512)):
                a, b = ps[0:64, csl], ps[64:128, csl]
                cc = cos_b[j][:, csl]
                ss = sin_b[j][:, csl]
                n = csl.stop - csl.start
                t1 = sp.tile([64, 512], BF, tag="rt1", name="rt1")[:, 0:n]
                t2 = sp.tile([64, 512], BF, tag="rt2", name="rt2")[:, 0:n]
                nc.vector.tensor_mul(t1, a, cc)
                nc.vector.tensor_mul(t2, b, ss)
                nc.gpsimd.tensor_sub(dst[0:64, csl], t1, t2)
                t3 = sp.tile([64, 512], BF, tag="rt3", name="rt3")[:, 0:n]
                t4 = sp.tile([64, 512], BF, tag="rt4", name="rt4")[:, 0:n]
                nc.vector.tensor_mul(t3, a, ss)
                nc.vector.tensor_mul(t4, b, cc)
                nc.gpsimd.tensor_add(dst[64:128, csl], t3, t4)

            qTb = {}  # j -> [128, HQ, 512]

            def v_evac(ps, j, csl=slice(0, 512)):
                vt = sp.tile([128, 512], BF, tag="vt", name="vt", bufs=2)
                nc.vector.tensor_copy(vt[:, csl], ps[:, csl])
                for tt in range(csl.start // 128, csl.stop // 128):
                    c = j * 4 + tt
                    tp = psT.tile([128, 128], BF, tag="trans", name="trans")
                    nc.tensor.transpose(
                        tp[:], vt[:, tt * 128 : (tt + 1) * 128], ident[:]
                    )
                    nc.vector.tensor_copy(vaug[:, c, 0:128], tp[:])

            # ---- projection emission -------------------------------------
            # A "group" is one psum accumulation over all 32 d-chunks for one
            # projection head (K, V, or Q h). Emitted as 4 units of 8 matmuls
            # so attention/exp work can interleave between units.
            def proj_group_units(j, kind, h=None, csl=slice(0, 512)):
                """Returns a list of unit callables; shared psum tile."""
                box = {}
                ncols = csl.stop - csl.start

                def unit(u):
                    def run():
                        if u == 0:
                            box["ps"] = psP.tile(
                                [128, 512], F32, tag="proj", name="proj"
                            )
                        ps = box["ps"]
                        if kind == "q":
                            w = wq_t[h]
                        elif kind == "k":
                            w = wk_t
                        else:
                            w = wv_t
                        for d in range(u * 8, u * 8 + 8):
                            nc.tensor.matmul(
                                ps[:, csl],
                                w[:, d, :],
                                xbd(j, d)[:, csl],
                                start=(d == 0),
                                stop=(d == ND - 1),
                            )
                        if u == 3:
                            if kind == "q":
                                rope(ps, qTb[j][:, h, :], j, csl)
                            elif kind == "k":
                                rope(ps, kT[:, j * 512 : (j + 1) * 512], j, csl)
                            else:
                                v_evac(ps, j, csl)

                    return run

                return [unit(u) for u in range(4)]

            def proj_units(j):
                """All projection units for block j (K, V, Q0..Q3)."""
                qTb[j] = blk_tile([128, HQ, 512], BF, "qTb")
                units = []
                units += proj_group_units(j, "k")
                units += proj_group_units(j, "v")
                for h in range(HQ):
                    units += proj_group_units(j, "q", h=h)
                return units

            def proj_units_block0():
                """Block 0 split: K/V/Q0 (in 256-col halves so compute can
                start before the x parts fully land) run inline; Q1-Q3 are
                returned as filler for attn(0) — they only gate scores h1-h3,
                so their wq/x DMA moves off the proj(0) critical path."""
                qTb[0] = blk_tile([128, HQ, 512], BF, "qTb")
                inline, deferred = [], []
                ha, hb = slice(0, 256), slice(256, 512)
                inline += proj_group_units(0, "k", csl=ha)
                inline += proj_group_units(0, "v", csl=ha)
                inline += proj_group_units(0, "q", h=0, csl=ha)
                inline += proj_group_units(0, "k", csl=hb)
                inline += proj_group_units(0, "v", csl=hb)
                inline += proj_group_units(0, "q", h=0, csl=hb)
                for h in range(1, HQ):
                    for csl in (ha, hb):
                        deferred += proj_group_units(0, "q", h=h, csl=csl)
                return inline, deferred

            # ---- output projection units (block jb) ----------------------
            attnTb = {}  # j -> [128, HQ, 512]
            stage = {}

            def d_units(jb):
                # Last block: per-n stores on the (idle by then) sync hw
                # queue so the final store's wire time overlaps D(NB-1)
                # compute instead of draining after the last matmul.
                last = jb == NB - 1
                units = []
                for tt in range(4):
                    for n in range(DIM // 512):
                        def run(tt=tt, n=n):
                            at = attnTb[jb]
                            if n == 0:
                                stage[jb] = op.tile(
                                    [128, DIM], BF, tag="stg", name="stg"
                                )
                            ps = psP.tile([128, 512], F32, tag="proj", name="proj")
                            for h in range(HQ):
                                nc.tensor.matmul(
                                    ps[:],
                                    at[:, h, tt * 128 : (tt + 1) * 128],
                                    wo_t[:, h, n * 512 : (n + 1) * 512],
                                    start=(h == 0),
                                    stop=(h == HQ - 1),
                                )
                            nc.vector.tensor_copy(
                                stage[jb][:, n * 512 : (n + 1) * 512], ps[:]
                            )
                            t = jb * 4 + tt
                            if last:
                                nsl = slice(n * 512, (n + 1) * 512)
                                nc.sync.dma_start(
                                    out=outD[t * 128 : (t + 1) * 128, nsl],
                                    in_=stage[jb][:, nsl],
                                )
                            elif n == DIM // 512 - 1:
                                nc.gpsimd.dma_start(
                                    out=outD[t * 128 : (t + 1) * 128, :],
                                    in_=stage[jb][:],
                                )

                        units.append(run)
                return units

            # ---- attention for block j with filler interleaving ----------
            def attn_block(j, filler, pre_pops=0):
                """filler: deque of (is_proj, fn). Returns leftover D units
                (proj units must drain inside their block: the next block's
                scores depend on them and sit earlier in PE program order)."""
                cmax = 4 * j + 4
                jsl = slice(j * 512, (j + 1) * 512)
                attnTb[j] = blk_tile([128, HQ, 512], BF, "attnTb")
                mask_alt = [nc.vector, nc.gpsimd]
                popped = 0
                target = len(filler)
                for h in range(HQ):
                    heads_left = HQ - h
                    quota = popped + (target - popped + heads_left - 1) // heads_left
                    # block 0: this head's deferred Q group must run before
                    # its scores
                    for _ in range(pre_pops):
                        if filler:
                            filler.popleft()[1]()
                            popped += 1
                    et = {}
                    for c in range(cmax):
                        ps = psS.tile([128, 512], F32, tag="scores", name="scores")
                        nc.tensor.matmul(
                            ps[:],
                            kT[:, c * 128 : (c + 1) * 128],
                            qTb[j][:, h, :],
                            start=True,
                            stop=True,
                        )
                        e = ep.tile([128, 512], BF, tag="et", name="et")
                        nc.scalar.activation(
                            e[:], ps[:], mybir.ActivationFunctionType.Exp, scale=SC
                        )
                        if c // 4 == j:
                            mask_alt[c % 2].tensor_mul(
                                e[:], e[:], dmask[:, c % 4, :]
                            )
                        et[c] = e

                        def av_tile(tt):
                            # AV tile tt gates on chunks <= 4j+tt only: run
                            # it right after its diagonal chunk so its
                            # matmuls cover the tail chunks' exp latency
                            t = 4 * j + tt
                            po = psO.tile([128, 129], F32, tag="oaug", name="oaug")
                            for i, cc in enumerate(range(t + 1)):
                                nc.tensor.matmul(
                                    po[:],
                                    et[cc][:, tt * 128 : (tt + 1) * 128],
                                    vaug[:, cc, :],
                                    start=(i == 0),
                                    stop=(i == t),
                                )
                            rs = sp.tile([128, 1], F32, tag="rs", name="rs", bufs=2)
                            nc.vector.reciprocal(rs[:], po[:, 128:129])
                            an = sp.tile([128, 128], BF, tag="an", name="an", bufs=2)
                            # ACT-side scale keeps the transpose chain off
                            # the busy DVE queue
                            nc.scalar.mul(an[:], po[:, 0:128], rs[:])
                            tp = psT.tile([128, 128], BF, tag="trans", name="trans")
                            nc.tensor.transpose(tp[:], an[:], ident[:])
                            nc.vector.tensor_copy(
                                attnTb[j][:, h, tt * 128 : (tt + 1) * 128], tp[:]
                            )

                        # interleave filler to keep the PE busy while ACT exps
                        want = quota * (c + 1) // cmax
                        while filler and popped < want:
                            filler.popleft()[1]()
                            popped += 1
                    while filler and popped < quota:
                        filler.popleft()[1]()
                        popped += 1
                    for tt in range(4):
                        av_tile(tt)
                while filler:
                    filler.popleft()[1]()
                return deque()

            # ---- main schedule -------------------------------------------
            inline0, deferred0 = proj_units_block0()
            for f in inline0:
                f()
            carry = deque()
            for j in range(NB):
                # DMA for block j+2 goes out at the start of attn(j)
                if j + 2 < NB:
                    load_cossin(j + 2, nc.sync)
                    alloc_x(j + 2)
                    for g in range(4):
                        load_x_part(j + 2, g, nc.sync)
                filler = deque(carry)
                if j == 0:
                    filler.extend((True, f) for f in deferred0)
                if j + 1 < NB:
                    filler.extend((True, f) for f in proj_units(j + 1))
                if j >= 1:
                    filler.extend((False, f) for f in d_units(j - 1))
                carry = attn_block(j, filler)
            for f in carry:
                f[1]()
            for f in d_units(NB - 1):
                f()

    nc.finalize()
    return nc


def _build_v1(S, mask_mode):
    """v2 phase-serial kernel, kept for the non-causal fallback paths."""
    import concourse.mybir as mybir
    import concourse.tile as tile
    from concourse import bacc

    dt = mybir.dt
    BF, F32 = dt.bfloat16, dt.float32
    NB = S // 512
    NT = S // 128
    ND = DIM // 128
    SC = 1.0 / math.sqrt(HD)

    nc = bacc.Bacc(None, target_bir_lowering=False)
    xT = nc.declare_dram_parameter("xT", [128, ND, S], BF, isOutput=False)
    wqT = nc.declare_dram_parameter("wqT", [HQ, 128, ND, 128], BF, isOutput=False)
    wkT = nc.declare_dram_parameter("wkT", [128, ND, 128], BF, isOutput=False)
    wvT = nc.declare_dram_parameter("wvT", [128, ND, 128], BF, isOutput=False)
    woT = nc.declare_dram_parameter("woT", [128, HQ, DIM], BF, isOutput=False)
    cosT = nc.declare_dram_parameter("cosT", [64, S], F32, isOutput=False)
    sinT = nc.declare_dram_parameter("sinT", [64, S], F32, isOutput=False)
    identD = nc.declare_dram_parameter("identD", [128, 128], BF, isOutput=False)
    if mask_mode == "full":
        maskTD = nc.declare_dram_parameter("maskTD", [NT, 128, S], F32, isOutput=False)
    outD = nc.declare_dram_parameter("out", [S, DIM], F32, isOutput=True)

    with tile.TileContext(nc) as tc:
        with (
            tc.tile_pool(name="persist", bufs=1) as pp,
            tc.tile_pool(name="xb", bufs=8) as xp,
            tc.tile_pool(name="wstream", bufs=2) as wp,
            tc.tile_pool(name="et", bufs=NT + 1) as ep,
            tc.tile_pool(name="small", bufs=2) as sp,
            tc.tile_pool(name="oev", bufs=5) as op,
            tc.tile_pool(name="ps_proj", bufs=2, space="PSUM") as psP,
            tc.tile_pool(name="ps_scores", bufs=3, space="PSUM") as psS,
            tc.tile_pool(name="ps_oaug", bufs=2, space="PSUM") as psO,
            tc.tile_pool(name="ps_trans", bufs=1, space="PSUM") as psT,
        ):
            cos = pp.tile([64, S], F32, tag="cos", name="cos")
            sin = pp.tile([64, S], F32, tag="sin", name="sin")
            nc.scalar.dma_start(out=cos[:], in_=cosT[:])
            nc.scalar.dma_start(out=sin[:], in_=sinT[:])
            ident = pp.tile([128, 128], BF, tag="ident", name="ident")
            nc.scalar.dma_start(out=ident[:], in_=identD[:])
            wk_t = wp.tile([128, ND, 128], BF, tag="wk", name="wk", bufs=1)
            wv_t = wp.tile([128, ND, 128], BF, tag="wv", name="wv", bufs=1)
            qT = [pp.tile([128, S], BF, tag=f"qT{h}", name=f"qT{h}") for h in range(HQ)]
            kT = pp.tile([128, S], BF, tag="kT", name="kT")
            attnT = [pp.tile([128, S], BF, tag=f"attnT{h}", name=f"attnT{h}") for h in range(HQ)]
            vaug = pp.tile([128, NT, 129], BF, tag="vaug", name="vaug")

            def rope(ps, dst, bsl):
                a, b = ps[0:64, :], ps[64:128, :]
                cc, ss = cos[:, bsl], sin[:, bsl]
                t1 = sp.tile([64, 512], F32, tag="rt1", name="rt1")
                t2 = sp.tile([64, 512], F32, tag="rt2", name="rt2")
                nc.vector.tensor_mul(t1[:], a, cc)
                nc.vector.tensor_mul(t2[:], b, ss)
                nc.vector.tensor_sub(dst[0:64, bsl], t1[:], t2[:])
                t3 = sp.tile([64, 512], F32, tag="rt1", name="rt1")
                t4 = sp.tile([64, 512], F32, tag="rt2", name="rt2")
                nc.vector.tensor_mul(t3[:], a, ss)
                nc.vector.tensor_mul(t4[:], b, cc)
                nc.vector.tensor_add(dst[64:128, bsl], t3[:], t4[:])

            DG = ND // 4
            for b in range(NB):
                bsl = slice(b * 512, (b + 1) * 512)
                wq_first = wp.tile([128, ND, 128], BF, tag="wqh", name="wqh")
                nc.scalar.dma_start(out=wq_first[:], in_=wqT[0])
                xbp = []
                for g in range(4):
                    xg = xp.tile([128, DG, 512], BF, tag="xb", name="xb")
                    nc.sync.dma_start(
                        out=xg[:], in_=xT[:, g * DG : (g + 1) * DG, bsl]
                    )
                    xbp.append(xg)
                if b == 0:
                    nc.sync.dma_start(out=wk_t[:], in_=wkT[:])
                    nc.sync.dma_start(out=wv_t[:], in_=wvT[:])

                def xbd(d):
                    return xbp[d // DG][:, d % DG, :]

                for h in range(HQ):
                    if h == 0:
                        wq_t = wq_first
                    else:
                        wq_t = wp.tile([128, ND, 128], BF, tag="wqh", name="wqh")
                        nc.scalar.dma_start(out=wq_t[:], in_=wqT[h])
                    ps = psP.tile([128, 512], F32, tag="proj", name="proj")
                    for d in range(ND):
                        nc.tensor.matmul(
                            ps[:], wq_t[:, d, :], xbd(d),
                            start=(d == 0), stop=(d == ND - 1),
                        )
                    rope(ps, qT[h], bsl)
                ps = psP.tile([128, 512], F32, tag="proj", name="proj")
                for d in range(ND):
                    nc.tensor.matmul(
                        ps[:], wk_t[:, d, :], xbd(d),
                        start=(d == 0), stop=(d == ND - 1),
                    )
                rope(ps, kT, bsl)
                ps = psP.tile([128, 512], F32, tag="proj", name="proj")
                for d in range(ND):
                    nc.tensor.matmul(
                        ps[:], wv_t[:, d, :], xbd(d),
                        start=(d == 0), stop=(d == ND - 1),
                    )
                vt = sp.tile([128, 512], BF, tag="vt", name="vt")
                nc.vector.tensor_copy(vt[:], ps[:])
                for tt in range(4):
                    c = b * 4 + tt
                    tp = psT.tile([128, 128], BF, tag="trans", name="trans")
                    nc.tensor.transpose(tp[:], vt[:, tt * 128 : (tt + 1) * 128], ident[:])
                    nc.vector.tensor_copy(vaug[:, c, 0:128], tp[:])
                    nc.vector.memset(vaug[:, c, 128:129], 1.0)

            for h in range(HQ):
                for j in range(NB):
                    jsl = slice(j * 512, (j + 1) * 512)
                    cmax = NT
                    et = {}
                    for c in range(cmax):
                        ps = psS.tile([128, 512], F32, tag="scores", name="scores")
                        nc.tensor.matmul(
                            ps[:],
                            kT[:, c * 128 : (c + 1) * 128],
                            qT[h][:, jsl],
                            start=True,
                            stop=True,
                        )
                        if mask_mode == "full":
                            mt = sp.tile([128, 512], F32, tag="mt", name="mt")
                            nc.sync.dma_start(out=mt[:], in_=maskTD[c, :, jsl])
                            nc.vector.scalar_tensor_tensor(
                                ps[:], ps[:], SC, mt[:],
                                op0=mybir.AluOpType.mult, op1=mybir.AluOpType.add,
                            )
                            sc_exp = 1.0
                        else:
                            sc_exp = SC
                        e = ep.tile([128, 512], BF, tag="et", name="et")
                        nc.scalar.activation(
                            e[:], ps[:], mybir.ActivationFunctionType.Exp, scale=sc_exp
                        )
                        et[c] = e
                    for tt in range(4):
                        t = 4 * j + tt
                        cs = list(range(cmax))
                        po = psO.tile([128, 129], F32, tag="oaug", name="oaug")
                        for i, c in enumerate(cs):
                            nc.tensor.matmul(
                                po[:],
                                et[c][:, tt * 128 : (tt + 1) * 128],
                                vaug[:, c, :],
                                start=(i == 0),
                                stop=(i == len(cs) - 1),
                            )
                        rs = sp.tile([128, 1], F32, tag="rs", name="rs")
                        nc.vector.reciprocal(rs[:], po[:, 128:129])
                        an = sp.tile([128, 128], BF, tag="an", name="an")
                        nc.vector.tensor_scalar_mul(an[:], po[:, 0:128], rs[:])
                        tp = psT.tile([128, 128], BF, tag="trans", name="trans")
                        nc.tensor.transpose(tp[:], an[:], ident[:])
                        nc.vector.tensor_copy(
                            attnT[h][:, t * 128 : (t + 1) * 128], tp[:]
                        )

            NDB = DIM // 512
            for n in range(NDB):
                wot = wp.tile([128, HQ, 512], BF, tag="wo", name="wo", bufs=3)
                nc.scalar.dma_start(out=wot[:], in_=woT[:, :, n * 512 : (n + 1) * 512])
                for t in range(NT):
                    ps = psP.tile([128, 512], F32, tag="proj", name="proj")
                    for h in range(HQ):
                        nc.tensor.matmul(
                            ps[:],
                            attnT[h][:, t * 128 : (t + 1) * 128],
                            wot[:, h, :],
                            start=(h == 0),
                            stop=(h == HQ - 1),
                        )
                    ov = op.tile([128, 512], F32, tag="ov", name="ov")
                    if t % 2 == 0:
                        nc.scalar.copy(ov[:], ps[:])
                    else:
                        nc.vector.tensor_copy(ov[:], ps[:])
                    st_eng = nc.sync if t % 2 == 0 else nc.scalar
                    st_eng.dma_start(
                        out=outD[t * 128 : (t + 1) * 128, n * 512 : (n + 1) * 512],
                        in_=ov[:],
                    )

    nc.finalize()
    return nc


def _prep_inputs(x, wq, wk, wv, wo, freqs_cos, freqs_sin, mask, S, mask_mode):
    """Host-side shard + layout prep. Returns list of in_maps (one per core)."""
    bf = ml_dtypes.bfloat16
    ND = DIM // 128
    NT = S // 128
    x2 = np.ascontiguousarray(x.reshape(S, DIM))
    if mask_mode == "causal":
        # block-major [128, NB, ND, 512]: xT[p, b, d, s'] = x[b*512+s', 128*d+p]
        NBb = S // 512
        xT = np.ascontiguousarray(
            x2.T.reshape(ND, 128, NBb, 512).transpose(1, 2, 0, 3)
        ).astype(bf)
        # block 0 half-major: x0T[p, half, d, s'] = x[half*256+s', 128*d+p]
        x0T = np.ascontiguousarray(
            x2[0:512].T.reshape(ND, 128, 2, 256).transpose(1, 2, 0, 3)
        ).astype(bf)
    else:
        # [128, ND, S] partition-major: xT[p, d, s] = x[s, 128*d + p]
        xT = np.ascontiguousarray(
            x2.T.reshape(ND, 128, S).transpose(1, 0, 2)
        ).astype(bf)
    cosT = np.ascontiguousarray(freqs_cos.T).astype(np.float32)
    sinT = np.ascontiguousarray(freqs_sin.T).astype(np.float32)
    ident = np.eye(128, dtype=bf)
    if mask_mode == "causal":
        r = np.arange(128)[:, None]
        col = np.arange(512)[None, :]
        dmask = np.stack(
            [(128 * p + r <= col) for p in range(4)], axis=1
        ).astype(bf)  # [128, 4, 512]
    elif mask_mode == "full":
        maskT = np.ascontiguousarray(mask.T).astype(np.float32).reshape(NT, 128, S)

    in_maps = []
    for m in range(NCORES):
        wq_l = wq[m * 512 : (m + 1) * 512]  # [512, 4096]
        wq_l = wq_l.reshape(HQ, HD, DIM)[:, _HEAD_PERM, :].reshape(512, DIM)
        wqT_l = np.ascontiguousarray(
            wq_l.T.reshape(ND, 128, HQ, 128).transpose(2, 1, 0, 3)
        ).astype(bf)
        wk_l = wk[m * 128 : (m + 1) * 128][_HEAD_PERM]
        wkT_l = np.ascontiguousarray(
            wk_l.T.reshape(ND, 128, 128).transpose(1, 0, 2)
        ).astype(bf)
        wv_l = wv[m * 128 : (m + 1) * 128]
        wvT_l = np.ascontiguousarray(
            wv_l.T.reshape(ND, 128, 128).transpose(1, 0, 2)
        ).astype(bf)
        woT_l = np.ascontiguousarray(
            wo[:, m * 512 : (m + 1) * 512].T.reshape(HQ, 128, DIM).transpose(1, 0, 2)
        ).astype(bf)
        im = {
            "xT": xT,
            "wqT": wqT_l,
            "wkT": wkT_l,
            "wvT": wvT_l,
            "woT": woT_l,
            "cosT": cosT,
            "sinT": sinT,
            "identD": ident,
        }
        if mask_mode == "causal":
            im["dmaskD"] = dmask
            im["x0T"] = x0T
        elif mask_mode == "full":
            im["maskTD"] = maskT
        in_maps.append(im)
    return in_maps


def _detect_mask_mode(mask):
    if not np.any(mask):
        return "none"
    S = mask.shape[0]
    causal = np.where(np.triu(np.ones((S, S), dtype=bool), k=1), -1e9, 0.0).astype(
        np.float32
    )
    if np.array_equal(mask, causal):
        return "causal"
    return "full"


def kernel(x, wq, wk, wv, wo, freqs_cos, freqs_sin, cache_k, cache_v, mask, start_pos):
    """Full inputs in, full output out. start_pos/caches are no-ops for these
    shapes (the reference's dynamic_update_slice clamps to a full overwrite)."""
    global LAST_EXEC_NS, LAST_RESULT
    from concourse.bass_utils import run_bass_kernel_spmd

    x = np.asarray(x, dtype=np.float32)
    B, S, _ = x.shape
    assert B == 1
    mask = np.asarray(mask, dtype=np.float32)
    mode = _detect_mask_mode(mask)
    if mode == "causal":
        nc = _build_causal(S)
    else:
        nc = _build_v1(S, mode)
    in_maps = _prep_inputs(
        x, np.asarray(wq, np.float32), np.asarray(wk, np.float32),
        np.asarray(wv, np.float32), np.asarray(wo, np.float32),
        np.asarray(freqs_cos, np.float32), np.asarray(freqs_sin, np.float32),
        mask, S, mode,
    )
    import os

    tmpdir = os.environ.get("BASS_KERNEL_TMPDIR") or None
    if tmpdir:
        os.makedirs(tmpdir, exist_ok=True)
    res = run_bass_kernel_spmd(nc, in_maps, list(range(NCORES)), tmpdir=tmpdir)
    LAST_EXEC_NS = res.exec_time_ns
    LAST_RESULT = res
    acc = np.zeros((S, DIM), dtype=np.float64)
    for r in res.results:
        acc += r["out"].astype(np.float64)
    return acc.astype(np.float32).reshape(1, S, DIM)


# revision 40
# speedup vs baseline: 1.0139x; 1.0139x over previous
"""Trainium2 Bass kernel: 32-head GQA attention prefill (Llama-style),
tensor-parallel over heads across 8 NeuronCores.

v3: block-pipelined schedule. Per s-block j the attention work (scores ->
exp on ACT -> AV) is interleaved on the PE with "filler" units taken from
the NEXT block's projections and the PREVIOUS block's output projection,
so the ACT engine's ~113us of exp work hides under PE matmul work instead
of serializing phase C (v2 lost ~35us there). Other v3 changes:
  - wq and wo fully resident in SBUF (no per-block wq reloads: -12MB HBM)
  - block-0 x loads split into halves across the sync+scalar hw DGE
    queues, first projection groups run at 256 cols -> lead-in ~9us vs 22
  - bf16 output partials staged into [128,4096] rows, 16 big stores on the
    Pool engine's software DGE (keeps sync/scalar queues for loads)
  - rope / mask / evacuation work split across DVE and the idle Pool
    engine

Math (per core m) unchanged from v2:
  local Q heads H = 4m..4m+3, local KV head = m.
  qT_h [hd,s] = wqT_h chunks @ xT (bf16, fp32 psum), RoPE fused on psum
  (head-dim de-interleaved host-side so rope pairs are partition halves).
  scoresT [sk,sq] = kT-chunk.T @ qT -> exp(./sqrt(128)) -> eT bf16,
  causal via chunk skipping + 0/1 mask-mul on diagonal chunks.
  out_aug [sq,129] = eT-chunks.T @ [v | 1]; attn = out_aug[:, :128]/rowsum,
  PE-transposed to attnT; partial_out = attnT-chunks.T @ woT.
  Host sums the 8 per-core bf16 partials (the "all-reduce after wo").
"""

import sys

sys.path.insert(0, "/opt/trn_rl_repo")

import math
from collections import deque

import ml_dtypes
import numpy as np

# bass_utils' trace branch imports antenv.axon_hooks, which some images
# lack; install a null shim so a stray BASS_TRACE env can't crash us.
try:
    import antenv.axon_hooks  # noqa: F401
except ImportError:
    import types as _types

    _hm = _types.ModuleType("antenv.axon_hooks")
    _hm._hook = None
    _hm.set_axon_ntff_profile_hook = lambda h: setattr(_hm, "_hook", h)
    _hm.get_axon_ntff_profile_hook = lambda: _hm._hook
    sys.modules["antenv.axon_hooks"] = _hm
    try:
        import antenv as _antenv

        _antenv.axon_hooks = _hm
    except ImportError:
        pass

DIM = 4096
NCORES = 8
HQ = 4  # local q heads per core
HD = 128


LAST_EXEC_NS = None
LAST_RESULT = None

_HEAD_PERM = np.concatenate([np.arange(0, HD, 2), np.arange(1, HD, 2)])


def _build_causal(S):
    """Fused causal kernel (v3). Returns nc."""
    import concourse.mybir as mybir
    import concourse.tile as tile
    from concourse import bacc

    dt = mybir.dt
    BF, F32 = dt.bfloat16, dt.float32
    NB = S // 512  # s blocks
    NT = S // 128  # s tiles == sk chunks
    ND = DIM // 128  # d chunks
    DG = ND // 4  # d-chunks per x part
    SC = 1.0 / math.sqrt(HD)

    nc = bacc.Bacc(None, target_bir_lowering=False)
    # x is block-major so each part-load is 8KB-contiguous per partition
    # (the [128, ND, S] layout produced 512B DMA packets that throttled the
    # early loads to ~100GB/s and starved proj(0))
    xT = nc.declare_dram_parameter("xT", [128, NB, ND, 512], BF, isOutput=False)
    # block 0 duplicated half-major so its 256-col half loads stay 4KB
    # contiguous per partition (halving the block-major layout would drop
    # DMA packets back to 512B)
    x0T = nc.declare_dram_parameter("x0T", [128, 2, ND, 256], BF, isOutput=False)
    wqT = nc.declare_dram_parameter("wqT", [HQ, 128, ND, 128], BF, isOutput=False)
    wkT = nc.declare_dram_parameter("wkT", [128, ND, 128], BF, isOutput=False)
    wvT = nc.declare_dram_parameter("wvT", [128, ND, 128], BF, isOutput=False)
    woT = nc.declare_dram_parameter("woT", [128, HQ, DIM], BF, isOutput=False)
    cosT = nc.declare_dram_parameter("cosT", [64, S], F32, isOutput=False)
    sinT = nc.declare_dram_parameter("sinT", [64, S], F32, isOutput=False)
    identD = nc.declare_dram_parameter("identD", [128, 128], BF, isOutput=False)
    dmaskD = nc.declare_dram_parameter("dmaskD", [128, 4, 512], BF, isOutput=False)
    outD = nc.declare_dram_parameter("out", [S, DIM], BF, isOutput=True)

    with tile.TileContext(nc) as tc:
        with (
            tc.tile_pool(name="persist", bufs=1) as pp,
            tc.tile_pool(name="xb", bufs=6) as xp,
            tc.tile_pool(name="blk", bufs=2) as bp,
            tc.tile_pool(name="et", bufs=NT + 1) as ep,
            tc.tile_pool(name="small", bufs=1) as sp,
            tc.tile_pool(name="stage", bufs=2) as op,
            tc.tile_pool(name="ps_proj", bufs=2, space="PSUM") as psP,
            tc.tile_pool(name="ps_scores", bufs=3, space="PSUM") as psS,
            tc.tile_pool(name="ps_oaug", bufs=2, space="PSUM") as psO,
            tc.tile_pool(name="ps_trans", bufs=1, space="PSUM") as psT,
        ):
            # ---- persistent tiles ----
            ident = pp.tile([128, 128], BF, tag="ident", name="ident")
            dmask = pp.tile([128, 4, 512], BF, tag="dmask", name="dmask")
            wk_t = pp.tile([128, ND, 128], BF, tag="wk", name="wk")
            wv_t = pp.tile([128, ND, 128], BF, tag="wv", name="wv")
            wq_t = [
                pp.tile([128, ND, 128], BF, tag=f"wq{h}", name=f"wq{h}")
                for h in range(HQ)
            ]
            wo_t = pp.tile([128, HQ, DIM], BF, tag="wo", name="wo")
            kT = pp.tile([128, S], BF, tag="kT", name="kT")
            vaug = pp.tile([128, NT, 129], BF, tag="vaug", name="vaug")
            nc.gpsimd.memset(vaug[:, :, 128:129], 1.0)

            # rotating per-block tiles
            def blk_tile(shape, dtp, tag):
                return bp.tile(shape, dtp, tag=tag, name=tag)

            # ---- initial DMA schedule ----
            # scalar (ACT hw DGE): small consts, half of x block0, wv, wo,
            #   cos/sin block1, x block1 p0/p1 (ACT is exp-busy later).
            # sync (SP hw DGE): other half of x block0, wq0-3, x b1 p2/p3;
            #   later blocks' x/cos/sin emitted at block boundaries.
            # gpsimd (Pool sw DGE): output stores only.
            nc.scalar.dma_start(out=wk_t[:], in_=wkT[:])
            nc.scalar.dma_start(out=ident[:], in_=identD[:])
            nc.scalar.dma_start(out=dmask[:], in_=dmaskD[:])

            cos_b = {}
            sin_b = {}

            def load_cossin(j, eng):
                jsl = slice(j * 512, (j + 1) * 512)
                cos_b[j] = blk_tile([64, 512], F32, "cosb")
                sin_b[j] = blk_tile([64, 512], F32, "sinb")
                eng.dma_start(out=cos_b[j][:], in_=cosT[:, jsl])
                eng.dma_start(out=sin_b[j][:], in_=sinT[:, jsl])

            load_cossin(0, nc.scalar)

            x_parts = {}  # (j, g) -> tile [128, DG, 512]

            def alloc_x(j):
                for g in range(4):
                    x_parts[(j, g)] = xp.tile(
                        [128, DG, 512], BF, tag="xb", name="xb"
                    )

            def load_x_part(j, g, eng, half=None):
                t = x_parts[(j, g)]
                if half is None:
                    eng.dma_start(out=t[:], in_=xT[:, j, g * DG : (g + 1) * DG, :])
                else:
                    eng.dma_start(
                        out=t[:, :, half * 256 : half * 256 + 256],
                        in_=x0T[:, half, g * DG : (g + 1) * DG, :],
                    )

            # block 0: halves split across the two hw queues so the first
            # 256-col projection groups can start ~9us in. Each queue is
            # ordered by first-use time: x halves, then wq (Q groups),
            # then block-1 x / cos/sin, then wo (first needed at D(0)).
            alloc_x(0)
            if NB > 1:
                alloc_x(1)
            # arrival order matches group consumption order:
            # K(a) V(a) K(b) V(b) inline, then Q0..Q3 as attn(0) filler
            # (sw-DGE loads on Pool would block the rope combines behind
            # descriptor generation — keep everything on the hw queues)
            for g in (0, 1):
                load_x_part(0, g, nc.sync, half=0)
            for g in (2, 3):
                load_x_part(0, g, nc.scalar, half=0)
            nc.scalar.dma_start(out=wv_t[:], in_=wvT[:])
            nc.sync.dma_start(out=wq_t[0][:], in_=wqT[0])
            for g in (0, 1):
                load_x_part(0, g, nc.sync, half=1)
            for g in (2, 3):
                load_x_part(0, g, nc.scalar, half=1)
            nc.sync.dma_start(out=wq_t[1][:], in_=wqT[1])
            nc.scalar.dma_start(out=wq_t[2][:], in_=wqT[2])
            nc.scalar.dma_start(out=wq_t[3][:], in_=wqT[3])
            if NB > 1:
                load_cossin(1, nc.scalar)
                load_x_part(1, 2, nc.sync)
                load_x_part(1, 3, nc.sync)
                load_x_part(1, 0, nc.sync)
                load_x_part(1, 1, nc.scalar)
            nc.scalar.dma_start(out=wo_t[:], in_=woT[:])

            def xbd(j, d):
                return x_parts[(j, d // DG)][:, d % DG, :]

            # ---- rope (psum [128,512] f32 -> dst bf16) ----
            # Pool can't read PSUM: the four products (psum reads) go on DVE,
            # the two SBUF-only combine steps on Pool.
            def rope(ps, dst, j, csl=slice(0, 512)):
                a, b = ps[0:64, csl], ps[64:128, csl]
                cc = cos_b[j][:, csl]
                ss = sin_b[j][:, csl]
                n = csl.stop - csl.start
                t1 = sp.tile([64, 512], BF, tag="rt1", name="rt1")[:, 0:n]
                t2 = sp.tile([64, 512], BF, tag="rt2", name="rt2")[:, 0:n]
                nc.vector.tensor_mul(t1, a, cc)
                nc.vector.tensor_mul(t2, b, ss)
                nc.gpsimd.tensor_sub(dst[0:64, csl], t1, t2)
                t3 = sp.tile([64, 512], BF, tag="rt3", name="rt3")[:, 0:n]
                t4 = sp.tile([64, 512], BF, tag="rt4", name="rt4")[:, 0:n]
                nc.vector.tensor_mul(t3, a, ss)
                nc.vector.tensor_mul(t4, b, cc)
                nc.gpsimd.tensor_add(dst[64:128, csl], t3, t4)

            qTb = {}  # j -> [128, HQ, 512]

            def v_evac(ps, j, csl=slice(0, 512)):
                vt = sp.tile([128, 512], BF, tag="vt", name="vt", bufs=2)
                nc.vector.tensor_copy(vt[:, csl], ps[:, csl])
                for tt in range(csl.start // 128, csl.stop // 128):
                    c = j * 4 + tt
                    tp = psT.tile([128, 128], BF, tag="trans", name="trans")
                    nc.tensor.transpose(
                        tp[:], vt[:, tt * 128 : (tt + 1) * 128], ident[:]
                    )
                    nc.vector.tensor_copy(vaug[:, c, 0:128], tp[:])

            # ---- projection emission -------------------------------------
            # A "group" is one psum accumulation over all 32 d-chunks for one
            # projection head (K, V, or Q h). Emitted as 4 units of 8 matmuls
            # so attention/exp work can interleave between units.
            def proj_group_units(j, kind, h=None, csl=slice(0, 512)):
                """Returns a list of unit callables; shared psum tile."""
                box = {}
                ncols = csl.stop - csl.start

                def unit(u):
                    def run():
                        if u == 0:
                            box["ps"] = psP.tile(
                                [128, 512], F32, tag="proj", name="proj"
                            )
                        ps = box["ps"]
                        if kind == "q":
                            w = wq_t[h]
                        elif kind == "k":
                            w = wk_t
                        else:
                            w = wv_t
                        for d in range(u * 8, u * 8 + 8):
                            nc.tensor.matmul(
                                ps[:, csl],
                                w[:, d, :],
                                xbd(j, d)[:, csl],
                                start=(d == 0),
                                stop=(d == ND - 1),
                            )
                        if u == 3:
                            if kind == "q":
                                rope(ps, qTb[j][:, h, :], j, csl)
                            elif kind == "k":
                                rope(ps, kT[:, j * 512 : (j + 1) * 512], j, csl)
                            else:
                                v_evac(ps, j, csl)

                    return run

                return [unit(u) for u in range(4)]

            def proj_units(j):
                """All projection units for block j (K, V, Q0..Q3)."""
                qTb[j] = blk_tile([128, HQ, 512], BF, "qTb")
                units = []
                units += proj_group_units(j, "k")
                units += proj_group_units(j, "v")
                for h in range(HQ):
                    units += proj_group_units(j, "q", h=h)
                return units

            def proj_units_block0():
                """Block 0 split: K/V/Q0 (in 256-col halves so compute can
                start before the x parts fully land) run inline; Q1-Q3 are
                returned as filler for attn(0) — they only gate scores h1-h3,
                so their wq/x DMA moves off the proj(0) critical path."""
                qTb[0] = blk_tile([128, HQ, 512], BF, "qTb")
                inline, deferred = [], []
                ha, hb = slice(0, 256), slice(256, 512)
                inline += proj_group_units(0, "k", csl=ha)
                inline += proj_group_units(0, "v", csl=ha)
                inline += proj_group_units(0, "q", h=0, csl=ha)
                inline += proj_group_units(0, "k", csl=hb)
                inline += proj_group_units(0, "v", csl=hb)
                inline += proj_group_units(0, "q", h=0, csl=hb)
                for h in range(1, HQ):
                    for csl in (ha, hb):
                        deferred += proj_group_units(0, "q", h=h, csl=csl)
                return inline, deferred

            # ---- output projection units (block jb) ----------------------
            attnTb = {}  # j -> [128, HQ, 512]
            stage = {}

            def d_units(jb):
                # Last block: per-n stores on the (idle by then) sync hw
                # queue so the final store's wire time overlaps D(NB-1)
                # compute instead of draining after the last matmul.
                last = jb == NB - 1
                units = []
                for tt in range(4):
                    for n in range(DIM // 512):
                        def run(tt=tt, n=n):
                            at = attnTb[jb]
                            if n == 0:
                                stage[jb] = op.tile(
                                    [128, DIM], BF, tag="stg", name="stg"
                                )
                            ps = psP.tile([128, 512], F32, tag="proj", name="proj")
                            for h in range(HQ):
                                nc.tensor.matmul(
                                    ps[:],
                                    at[:, h, tt * 128 : (tt + 1) * 128],
                                    wo_t[:, h, n * 512 : (n + 1) * 512],
                                    start=(h == 0),
                                    stop=(h == HQ - 1),
                                )
                            nc.vector.tensor_copy(
                                stage[jb][:, n * 512 : (n + 1) * 512], ps[:]
                            )
                            t = jb * 4 + tt
                            if last:
                                nsl = slice(n * 512, (n + 1) * 512)
                                nc.sync.dma_start(
                                    out=outD[t * 128 : (t + 1) * 128, nsl],
                                    in_=stage[jb][:, nsl],
                                )
                            elif n == DIM // 512 - 1:
                                nc.gpsimd.dma_start(
                                    out=outD[t * 128 : (t + 1) * 128, :],
                                    in_=stage[jb][:],
                                )

                        units.append(run)
                return units

            # ---- attention for block j with filler interleaving ----------
            def attn_block(j, filler, pre_pops=0):
                """filler: deque of (is_proj, fn). Returns leftover D units
                (proj units must drain inside their block: the next block's
                scores depend on them and sit earlier in PE program order)."""
                cmax = 4 * j + 4
                jsl = slice(j * 512, (j + 1) * 512)
                attnTb[j] = blk_tile([128, HQ, 512], BF, "attnTb")
                mask_alt = [nc.vector, nc.gpsimd]
                popped = 0
                target = len(filler)
                for h in range(HQ):
                    heads_left = HQ - h
                    quota = popped + (target - popped + heads_left - 1) // heads_left
                    # block 0: this head's deferred Q group must run before
                    # its scores
                    for _ in range(pre_pops):
                        if filler:
                            filler.popleft()[1]()
                            popped += 1
                    et = {}
                    for c in range(cmax):
                        ps = psS.tile([128, 512], F32, tag="scores", name="scores")
                        nc.tensor.matmul(
                            ps[:],
                            kT[:, c * 128 : (c + 1) * 128],
                            qTb[j][:, h, :],
                            start=True,
                            stop=True,
                        )
                        e = ep.tile([128, 512], BF, tag="et", name="et")
                        nc.scalar.activation(
                            e[:], ps[:], mybir.ActivationFunctionType.Exp, scale=SC
                        )
                        if c // 4 == j:
                            mask_alt[c % 2].tensor_mul(
                                e[:], e[:], dmask[:, c % 4, :]
                            )
                        et[c] = e

                        def av_tile(tt):
                            # AV tile tt gates on chunks <= 4j+tt only: run
                            # it right after its diagonal chunk so its
                            # matmuls cover the tail chunks' exp latency
                            t = 4 * j + tt
                            po = psO.tile([128, 129], F32, tag="oaug", name="oaug")
                            for i, cc in enumerate(range(t + 1)):
                                nc.tensor.matmul(
                                    po[:],
                                    et[cc][:, tt * 128 : (tt + 1) * 128],
                                    vaug[:, cc, :],
                                    start=(i == 0),
                                    stop=(i == t),
                                )
                            rs = sp.tile([128, 1], F32, tag="rs", name="rs", bufs=2)
                            nc.vector.reciprocal(rs[:], po[:, 128:129])
                            an = sp.tile([128, 128], BF, tag="an", name="an", bufs=2)
                            # ACT-side scale keeps the transpose chain off
                            # the busy DVE queue
                            nc.scalar.mul(an[:], po[:, 0:128], rs[:])
                            tp = psT.tile([128, 128], BF, tag="trans", name="trans")
                            nc.tensor.transpose(tp[:], an[:], ident[:])
                            nc.vector.tensor_copy(
                                attnTb[j][:, h, tt * 128 : (tt + 1) * 128], tp[:]
                            )

                        # interleave filler to keep the PE busy while ACT exps
                        want = quota * (c + 1) // cmax
                        while filler and popped < want:
                            filler.popleft()[1]()
                            popped += 1
                    while filler and popped < quota:
                        filler.popleft()[1]()
                        popped += 1
                    for tt in range(4):
                        av_tile(tt)
                while filler:
                    filler.popleft()[1]()
                return deque()

            # ---- main schedule -------------------------------------------
            inline0, deferred0 = proj_units_block0()
            for f in inline0:
                f()
            carry = deque()
            for j in range(NB):
                # DMA for block j+2 goes out at the start of attn(j)
                if j + 2 < NB:
                    load_cossin(j + 2, nc.sync)
                    alloc_x(j + 2)
                    for g in range(4):
                        load_x_part(j + 2, g, nc.sync)
                filler = deque(carry)
                if j == 0:
                    filler.extend((True, f) for f in deferred0)
                if j + 1 < NB:
                    filler.extend((True, f) for f in proj_units(j + 1))
                if j >= 1:
                    filler.extend((False, f) for f in d_units(j - 1))
                carry = attn_block(j, filler)
            for f in carry:
                f[1]()
            for f in d_units(NB - 1):
                f()

    nc.finalize()
    return nc


def _build_v1(S, mask_mode):
    """v2 phase-serial kernel, kept for the non-causal fallback paths."""
    import concourse.mybir as mybir
    import concourse.tile as tile
    from concourse import bacc

    dt = mybir.dt
    BF, F32 = dt.bfloat16, dt.float32
    NB = S // 512
    NT = S // 128
    ND = DIM // 128
    SC = 1.0 / math.sqrt(HD)

    nc = bacc.Bacc(None, target_bir_lowering=False)
    xT = nc.declare_dram_parameter("xT", [128, ND, S], BF, isOutput=False)
    wqT = nc.declare_dram_parameter("wqT", [HQ, 128, ND, 128], BF, isOutput=False)
    wkT = nc.declare_dram_parameter("wkT", [128, ND, 128], BF, isOutput=False)
    wvT = nc.declare_dram_parameter("wvT", [128, ND, 128], BF, isOutput=False)
    woT = nc.declare_dram_parameter("woT", [128, HQ, DIM], BF, isOutput=False)
    cosT = nc.declare_dram_parameter("cosT", [64, S], F32, isOutput=False)
    sinT = nc.declare_dram_parameter("sinT", [64, S], F32, isOutput=False)
    identD = nc.declare_dram_parameter("identD", [128, 128], BF, isOutput=False)
    if mask_mode == "full":
        maskTD = nc.declare_dram_parameter("maskTD", [NT, 128, S], F32, isOutput=False)
    outD = nc.declare_dram_parameter("out", [S, DIM], F32, isOutput=True)

    with tile.TileContext(nc) as tc:
        with (
            tc.tile_pool(name="persist", bufs=1) as pp,
            tc.tile_pool(name="xb", bufs=8) as xp,
            tc.tile_pool(name="wstream", bufs=2) as wp,
            tc.tile_pool(name="et", bufs=NT + 1) as ep,
            tc.tile_pool(name="small", bufs=2) as sp,
            tc.tile_pool(name="oev", bufs=5) as op,
            tc.tile_pool(name="ps_proj", bufs=2, space="PSUM") as psP,
            tc.tile_pool(name="ps_scores", bufs=3, space="PSUM") as psS,
            tc.tile_pool(name="ps_oaug", bufs=2, space="PSUM") as psO,
            tc.tile_pool(name="ps_trans", bufs=1, space="PSUM") as psT,
        ):
            cos = pp.tile([64, S], F32, tag="cos", name="cos")
            sin = pp.tile([64, S], F32, tag="sin", name="sin")
            nc.scalar.dma_start(out=cos[:], in_=cosT[:])
            nc.scalar.dma_start(out=sin[:], in_=sinT[:])
            ident = pp.tile([128, 128], BF, tag="ident", name="ident")
            nc.scalar.dma_start(out=ident[:], in_=identD[:])
            wk_t = wp.tile([128, ND, 128], BF, tag="wk", name="wk", bufs=1)
            wv_t = wp.tile([128, ND, 128], BF, tag="wv", name="wv", bufs=1)
            qT = [pp.tile([128, S], BF, tag=f"qT{h}", name=f"qT{h}") for h in range(HQ)]
            kT = pp.tile([128, S], BF, tag="kT", name="kT")
            attnT = [pp.tile([128, S], BF, tag=f"attnT{h}", name=f"attnT{h}") for h in range(HQ)]
            vaug = pp.tile([128, NT, 129], BF, tag="vaug", name="vaug")

            def rope(ps, dst, bsl):
                a, b = ps[0:64, :], ps[64:128, :]
                cc, ss = cos[:, bsl], sin[:, bsl]
                t1 = sp.tile([64, 512], F32, tag="rt1", name="rt1")
                t2 = sp.tile([64, 512], F32, tag="rt2", name="rt2")
                nc.vector.tensor_mul(t1[:], a, cc)
                nc.vector.tensor_mul(t2[:], b, ss)
                nc.vector.tensor_sub(dst[0:64, bsl], t1[:], t2[:])
                t3 = sp.tile([64, 512], F32, tag="rt1", name="rt1")
                t4 = sp.tile([64, 512], F32, tag="rt2", name="rt2")
                nc.vector.tensor_mul(t3[:], a, ss)
                nc.vector.tensor_mul(t4[:], b, cc)
                nc.vector.tensor_add(dst[64:128, bsl], t3[:], t4[:])

            DG = ND // 4
            for b in range(NB):
                bsl = slice(b * 512, (b + 1) * 512)
                wq_first = wp.tile([128, ND, 128], BF, tag="wqh", name="wqh")
                nc.scalar.dma_start(out=wq_first[:], in_=wqT[0])
                xbp = []
                for g in range(4):
                    xg = xp.tile([128, DG, 512], BF, tag="xb", name="xb")
                    nc.sync.dma_start(
                        out=xg[:], in_=xT[:, g * DG : (g + 1) * DG, bsl]
                    )
                    xbp.append(xg)
                if b == 0:
                    nc.sync.dma_start(out=wk_t[:], in_=wkT[:])
                    nc.sync.dma_start(out=wv_t[:], in_=wvT[:])

                def xbd(d):
                    return xbp[d // DG][:, d % DG, :]

                for h in range(HQ):
                    if h == 0:
                        wq_t = wq_first
                    else:
                        wq_t = wp.tile([128, ND, 128], BF, tag="wqh", name="wqh")
                        nc.scalar.dma_start(out=wq_t[:], in_=wqT[h])
                    ps = psP.tile([128, 512], F32, tag="proj", name="proj")
                    for d in range(ND):
                        nc.tensor.matmul(
                            ps[:], wq_t[:, d, :], xbd(d),
                            start=(d == 0), stop=(d == ND - 1),
                        )
                    rope(ps, qT[h], bsl)
                ps = psP.tile([128, 512], F32, tag="proj", name="proj")
                for d in range(ND):
                    nc.tensor.matmul(
                        ps[:], wk_t[:, d, :], xbd(d),
                        start=(d == 0), stop=(d == ND - 1),
                    )
                rope(ps, kT, bsl)
                ps = psP.tile([128, 512], F32, tag="proj", name="proj")
                for d in range(ND):
                    nc.tensor.matmul(
                        ps[:], wv_t[:, d, :], xbd(d),
                        start=(d == 0), stop=(d == ND - 1),
                    )
                vt = sp.tile([128, 512], BF, tag="vt", name="vt")
                nc.vector.tensor_copy(vt[:], ps[:])
                for tt in range(4):
                    c = b * 4 + tt
                    tp = psT.tile([128, 128], BF, tag="trans", name="trans")
                    nc.tensor.transpose(tp[:], vt[:, tt * 128 : (tt + 1) * 128], ident[:])
                    nc.vector.tensor_copy(vaug[:, c, 0:128], tp[:])
                    nc.vector.memset(vaug[:, c, 128:129], 1.0)

            for h in range(HQ):
                for j in range(NB):
                    jsl = slice(j * 512, (j + 1) * 512)
                    cmax = NT
                    et = {}
                    for c in range(cmax):
                        ps = psS.tile([128, 512], F32, tag="scores", name="scores")
                        nc.tensor.matmul(
                            ps[:],
                            kT[:, c * 128 : (c + 1) * 128],
                            qT[h][:, jsl],
                            start=True,
                            stop=True,
                        )
                        if mask_mode == "full":
                            mt = sp.tile([128, 512], F32, tag="mt", name="mt")
                            nc.sync.dma_start(out=mt[:], in_=maskTD[c, :, jsl])
                            nc.vector.scalar_tensor_tensor(
                                ps[:], ps[:], SC, mt[:],
                                op0=mybir.AluOpType.mult, op1=mybir.AluOpType.add,
                            )
                            sc_exp = 1.0
                        else:
                            sc_exp = SC
                        e = ep.tile([128, 512], BF, tag="et", name="et")
                        nc.scalar.activation(
                            e[:], ps[:], mybir.ActivationFunctionType.Exp, scale=sc_exp
                        )
                        et[c] = e
                    for tt in range(4):
                        t = 4 * j + tt
                        cs = list(range(cmax))
                        po = psO.tile([128, 129], F32, tag="oaug", name="oaug")
                        for i, c in enumerate(cs):
                            nc.tensor.matmul(
                                po[:],
                                et[c][:, tt * 128 : (tt + 1) * 128],
                                vaug[:, c, :],
                                start=(i == 0),
                                stop=(i == len(cs) - 1),
                            )
                        rs = sp.tile([128, 1], F32, tag="rs", name="rs")
                        nc.vector.reciprocal(rs[:], po[:, 128:129])
                        an = sp.tile([128, 128], BF, tag="an", name="an")
                        nc.vector.tensor_scalar_mul(an[:], po[:, 0:128], rs[:])
                        tp = psT.tile([128, 128], BF, tag="trans", name="trans")
                        nc.tensor.transpose(tp[:], an[:], ident[:])
                        nc.vector.tensor_copy(
                            attnT[h][:, t * 128 : (t + 1) * 128], tp[:]
                        )

            NDB = DIM // 512
            for n in range(NDB):
                wot = wp.tile([128, HQ, 512], BF, tag="wo", name="wo", bufs=3)
                nc.scalar.dma_start(out=wot[:], in_=woT[:, :, n * 512 : (n + 1) * 512])
                for t in range(NT):
                    ps = psP.tile([128, 512], F32, tag="proj", name="proj")
                    for h in range(HQ):
                        nc.tensor.matmul(
                            ps[:],
                            attnT[h][:, t * 128 : (t + 1) * 128],
                            wot[:, h, :],
                            start=(h == 0),
                            stop=(h == HQ - 1),
                        )
                    ov = op.tile([128, 512], F32, tag="ov", name="ov")
                    if t % 2 == 0:
                        nc.scalar.copy(ov[:], ps[:])
                    else:
                        nc.vector.tensor_copy(ov[:], ps[:])
                    st_eng = nc.sync if t % 2 == 0 else nc.scalar
                    st_eng.dma_start(
                        out=outD[t * 128 : (t + 1) * 128, n * 512 : (n + 1) * 512],
                        in_=ov[:],
                    )

    nc.finalize()
    return nc


def _prep_inputs(x, wq, wk, wv, wo, freqs_cos, freqs_sin, mask, S, mask_mode):
    """Host-side shard + layout prep. Returns list of in_maps (one per core)."""
    bf = ml_dtypes.bfloat16
    ND = DIM // 128
    NT = S // 128
    x2 = np.ascontiguousarray(x.reshape(S, DIM))
    if mask_mode == "causal":
        # block-major [128, NB, ND, 512]: xT[p, b, d, s'] = x[b*512+s', 128*d+p]
        NBb = S // 512
        xT = np.ascontiguousarray(
            x2.T.reshape(ND, 128, NBb, 512).transpose(1, 2, 0, 3)
        ).astype(bf)
        # block 0 half-major: x0T[p, half, d, s'] = x[half*256+s', 128*d+p]
        x0T = np.ascontiguousarray(
            x2[0:512].T.reshape(ND, 128, 2, 256).transpose(1, 2, 0, 3)
        ).astype(bf)
    else:
        # [128, ND, S] partition-major: xT[p, d, s] = x[s, 128*d + p]
        xT = np.ascontiguousarray(
            x2.T.reshape(ND, 128, S).transpose(1, 0, 2)
        ).astype(bf)
    cosT = np.ascontiguousarray(freqs_cos.T).astype(np.float32)
    sinT = np.ascontiguousarray(freqs_sin.T).astype(np.float32)
    ident = np.eye(128, dtype=bf)
    if mask_mode == "causal":
        r = np.arange(128)[:, None]
        col = np.arange(512)[None, :]
        dmask = np.stack(
            [(128 * p + r <= col) for p in range(4)], axis=1
        ).astype(bf)  # [128, 4, 512]
    elif mask_mode == "full":
        maskT = np.ascontiguousarray(mask.T).astype(np.float32).reshape(NT, 128, S)

    in_maps = []
    for m in range(NCORES):
        wq_l = wq[m * 512 : (m + 1) * 512]  # [512, 4096]
        wq_l = wq_l.reshape(HQ, HD, DIM)[:, _HEAD_PERM, :].reshape(512, DIM)
        wqT_l = np.ascontiguousarray(
            wq_l.T.reshape(ND, 128, HQ, 128).transpose(2, 1, 0, 3)
        ).astype(bf)
        wk_l = wk[m * 128 : (m + 1) * 128][_HEAD_PERM]
        wkT_l = np.ascontiguousarray(
            wk_l.T.reshape(ND, 128, 128).transpose(1, 0, 2)
        ).astype(bf)
        wv_l = wv[m * 128 : (m + 1) * 128]
        wvT_l = np.ascontiguousarray(
            wv_l.T.reshape(ND, 128, 128).transpose(1, 0, 2)
        ).astype(bf)
        woT_l = np.ascontiguousarray(
            wo[:, m * 512 : (m + 1) * 512].T.reshape(HQ, 128, DIM).transpose(1, 0, 2)
        ).astype(bf)
        im = {
            "xT": xT,
            "wqT": wqT_l,
            "wkT": wkT_l,
            "wvT": wvT_l,
            "woT": woT_l,
            "cosT": cosT,
            "sinT": sinT,
            "identD": ident,
        }
        if mask_mode == "causal":
            im["dmaskD"] = dmask
            im["x0T"] = x0T
        elif mask_mode == "full":
            im["maskTD"] = maskT
        in_maps.append(im)
    return in_maps


def _detect_mask_mode(mask):
    if not np.any(mask):
        return "none"
    S = mask.shape[0]
    causal = np.where(np.triu(np.ones((S, S), dtype=bool), k=1), -1e9, 0.0).astype(
        np.float32
    )
    if np.array_equal(mask, causal):
        return "causal"
    return "full"


def kernel(x, wq, wk, wv, wo, freqs_cos, freqs_sin, cache_k, cache_v, mask, start_pos):
    """Full inputs in, full output out. start_pos/caches are no-ops for these
    shapes (the reference's dynamic_update_slice clamps to a full overwrite)."""
    global LAST_EXEC_NS, LAST_RESULT
    from concourse.bass_utils import run_bass_kernel_spmd

    x = np.asarray(x, dtype=np.float32)
    B, S, _ = x.shape
    assert B == 1
    mask = np.asarray(mask, dtype=np.float32)
    mode = _detect_mask_mode(mask)
    if mode == "causal":
        nc = _build_causal(S)
    else:
        nc = _build_v1(S, mode)
    in_maps = _prep_inputs(
        x, np.asarray(wq, np.float32), np.asarray(wk, np.float32),
        np.asarray(wv, np.float32), np.asarray(wo, np.float32),
        np.asarray(freqs_cos, np.float32), np.asarray(freqs_sin, np.float32),
        mask, S, mode,
    )
    import os

    tmpdir = os.environ.get("BASS_KERNEL_TMPDIR") or None
    if tmpdir:
        os.makedirs(tmpdir, exist_ok=True)
    res = run_bass_kernel_spmd(nc, in_maps, list(range(NCORES)), tmpdir=tmpdir)
    LAST_EXEC_NS = res.exec_time_ns
    LAST_RESULT = res
    acc = np.zeros((S, DIM), dtype=np.float64)
    for r in res.results:
        acc += r["out"].astype(np.float64)
    return acc.astype(np.float32).reshape(1, S, DIM)


# revision 41
# speedup vs baseline: 1.0203x; 1.0063x over previous
"""Trainium2 Bass kernel: 32-head GQA attention prefill (Llama-style),
tensor-parallel over heads across 8 NeuronCores.

v3: block-pipelined schedule. Per s-block j the attention work (scores ->
exp on ACT -> AV) is interleaved on the PE with "filler" units taken from
the NEXT block's projections and the PREVIOUS block's output projection,
so the ACT engine's ~113us of exp work hides under PE matmul work instead
of serializing phase C (v2 lost ~35us there). Other v3 changes:
  - wq and wo fully resident in SBUF (no per-block wq reloads: -12MB HBM)
  - block-0 x loads split into halves across the sync+scalar hw DGE
    queues, first projection groups run at 256 cols -> lead-in ~9us vs 22
  - bf16 output partials staged into [128,4096] rows, 16 big stores on the
    Pool engine's software DGE (keeps sync/scalar queues for loads)
  - rope / mask / evacuation work split across DVE and the idle Pool
    engine

Math (per core m) unchanged from v2:
  local Q heads H = 4m..4m+3, local KV head = m.
  qT_h [hd,s] = wqT_h chunks @ xT (bf16, fp32 psum), RoPE fused on psum
  (head-dim de-interleaved host-side so rope pairs are partition halves).
  scoresT [sk,sq] = kT-chunk.T @ qT -> exp(./sqrt(128)) -> eT bf16,
  causal via chunk skipping + 0/1 mask-mul on diagonal chunks.
  out_aug [sq,129] = eT-chunks.T @ [v | 1]; attn = out_aug[:, :128]/rowsum,
  PE-transposed to attnT; partial_out = attnT-chunks.T @ woT.
  Host sums the 8 per-core bf16 partials (the "all-reduce after wo").
"""

import sys

sys.path.insert(0, "/opt/trn_rl_repo")

import math
from collections import deque

import ml_dtypes
import numpy as np

# bass_utils' trace branch imports antenv.axon_hooks, which some images
# lack; install a null shim so a stray BASS_TRACE env can't crash us.
try:
    import antenv.axon_hooks  # noqa: F401
except ImportError:
    import types as _types

    _hm = _types.ModuleType("antenv.axon_hooks")
    _hm._hook = None
    _hm.set_axon_ntff_profile_hook = lambda h: setattr(_hm, "_hook", h)
    _hm.get_axon_ntff_profile_hook = lambda: _hm._hook
    sys.modules["antenv.axon_hooks"] = _hm
    try:
        import antenv as _antenv

        _antenv.axon_hooks = _hm
    except ImportError:
        pass

DIM = 4096
NCORES = 8
HQ = 4  # local q heads per core
HD = 128


LAST_EXEC_NS = None
LAST_RESULT = None

_HEAD_PERM = np.concatenate([np.arange(0, HD, 2), np.arange(1, HD, 2)])


def _build_causal(S):
    """Fused causal kernel (v3). Returns nc."""
    import concourse.mybir as mybir
    import concourse.tile as tile
    from concourse import bacc

    dt = mybir.dt
    BF, F32 = dt.bfloat16, dt.float32
    NB = S // 512  # s blocks
    NT = S // 128  # s tiles == sk chunks
    ND = DIM // 128  # d chunks
    DG = ND // 4  # d-chunks per x part
    SC = 1.0 / math.sqrt(HD)

    nc = bacc.Bacc(None, target_bir_lowering=False)
    # x is block-major so each part-load is 8KB-contiguous per partition
    # (the [128, ND, S] layout produced 512B DMA packets that throttled the
    # early loads to ~100GB/s and starved proj(0))
    xT = nc.declare_dram_parameter("xT", [128, NB, ND, 512], BF, isOutput=False)
    # block 0 duplicated half-major so its 256-col half loads stay 4KB
    # contiguous per partition (halving the block-major layout would drop
    # DMA packets back to 512B)
    x0T = nc.declare_dram_parameter("x0T", [128, 2, ND, 256], BF, isOutput=False)
    wqT = nc.declare_dram_parameter("wqT", [HQ, 128, ND, 128], BF, isOutput=False)
    wkT = nc.declare_dram_parameter("wkT", [128, ND, 128], BF, isOutput=False)
    wvT = nc.declare_dram_parameter("wvT", [128, ND, 128], BF, isOutput=False)
    woT = nc.declare_dram_parameter("woT", [128, HQ, DIM], BF, isOutput=False)
    cosT = nc.declare_dram_parameter("cosT", [64, S], F32, isOutput=False)
    sinT = nc.declare_dram_parameter("sinT", [64, S], F32, isOutput=False)
    identD = nc.declare_dram_parameter("identD", [128, 128], BF, isOutput=False)
    dmaskD = nc.declare_dram_parameter("dmaskD", [128, 4, 512], BF, isOutput=False)
    outD = nc.declare_dram_parameter("out", [S, DIM], BF, isOutput=True)

    with tile.TileContext(nc) as tc:
        with (
            tc.tile_pool(name="persist", bufs=1) as pp,
            tc.tile_pool(name="xb", bufs=6) as xp,
            tc.tile_pool(name="blk", bufs=2) as bp,
            tc.tile_pool(name="et", bufs=NT + 1) as ep,
            tc.tile_pool(name="small", bufs=1) as sp,
            tc.tile_pool(name="stage", bufs=2) as op,
            tc.tile_pool(name="ps_proj", bufs=3, space="PSUM") as psP,
            tc.tile_pool(name="ps_scores", bufs=2, space="PSUM") as psS,
            tc.tile_pool(name="ps_oaug", bufs=2, space="PSUM") as psO,
            tc.tile_pool(name="ps_trans", bufs=1, space="PSUM") as psT,
        ):
            # ---- persistent tiles ----
            ident = pp.tile([128, 128], BF, tag="ident", name="ident")
            dmask = pp.tile([128, 4, 512], BF, tag="dmask", name="dmask")
            wk_t = pp.tile([128, ND, 128], BF, tag="wk", name="wk")
            wv_t = pp.tile([128, ND, 128], BF, tag="wv", name="wv")
            wq_t = [
                pp.tile([128, ND, 128], BF, tag=f"wq{h}", name=f"wq{h}")
                for h in range(HQ)
            ]
            wo_t = pp.tile([128, HQ, DIM], BF, tag="wo", name="wo")
            kT = pp.tile([128, S], BF, tag="kT", name="kT")
            vaug = pp.tile([128, NT, 129], BF, tag="vaug", name="vaug")
            nc.gpsimd.memset(vaug[:, :, 128:129], 1.0)

            # rotating per-block tiles
            def blk_tile(shape, dtp, tag):
                return bp.tile(shape, dtp, tag=tag, name=tag)

            # ---- initial DMA schedule ----
            # scalar (ACT hw DGE): small consts, half of x block0, wv, wo,
            #   cos/sin block1, x block1 p0/p1 (ACT is exp-busy later).
            # sync (SP hw DGE): other half of x block0, wq0-3, x b1 p2/p3;
            #   later blocks' x/cos/sin emitted at block boundaries.
            # gpsimd (Pool sw DGE): output stores only.
            nc.scalar.dma_start(out=wk_t[:], in_=wkT[:])
            nc.scalar.dma_start(out=ident[:], in_=identD[:])
            nc.scalar.dma_start(out=dmask[:], in_=dmaskD[:])

            cos_b = {}
            sin_b = {}

            def load_cossin(j, eng):
                jsl = slice(j * 512, (j + 1) * 512)
                cos_b[j] = blk_tile([64, 512], F32, "cosb")
                sin_b[j] = blk_tile([64, 512], F32, "sinb")
                eng.dma_start(out=cos_b[j][:], in_=cosT[:, jsl])
                eng.dma_start(out=sin_b[j][:], in_=sinT[:, jsl])

            load_cossin(0, nc.scalar)

            x_parts = {}  # (j, g) -> tile [128, DG, 512]

            def alloc_x(j):
                for g in range(4):
                    x_parts[(j, g)] = xp.tile(
                        [128, DG, 512], BF, tag="xb", name="xb"
                    )

            def load_x_part(j, g, eng, half=None):
                t = x_parts[(j, g)]
                if half is None:
                    eng.dma_start(out=t[:], in_=xT[:, j, g * DG : (g + 1) * DG, :])
                else:
                    eng.dma_start(
                        out=t[:, :, half * 256 : half * 256 + 256],
                        in_=x0T[:, half, g * DG : (g + 1) * DG, :],
                    )

            # block 0: halves split across the two hw queues so the first
            # 256-col projection groups can start ~9us in. Each queue is
            # ordered by first-use time: x halves, then wq (Q groups),
            # then block-1 x / cos/sin, then wo (first needed at D(0)).
            alloc_x(0)
            if NB > 1:
                alloc_x(1)
            # arrival order matches group consumption order:
            # K(a) V(a) K(b) V(b) inline, then Q0..Q3 as attn(0) filler
            # (sw-DGE loads on Pool would block the rope combines behind
            # descriptor generation — keep everything on the hw queues)
            for g in (0, 1):
                load_x_part(0, g, nc.sync, half=0)
            for g in (2, 3):
                load_x_part(0, g, nc.scalar, half=0)
            nc.scalar.dma_start(out=wv_t[:], in_=wvT[:])
            nc.sync.dma_start(out=wq_t[0][:], in_=wqT[0])
            for g in (0, 1):
                load_x_part(0, g, nc.sync, half=1)
            for g in (2, 3):
                load_x_part(0, g, nc.scalar, half=1)
            nc.sync.dma_start(out=wq_t[1][:], in_=wqT[1])
            nc.scalar.dma_start(out=wq_t[2][:], in_=wqT[2])
            nc.scalar.dma_start(out=wq_t[3][:], in_=wqT[3])
            if NB > 1:
                load_cossin(1, nc.scalar)
                load_x_part(1, 2, nc.sync)
                load_x_part(1, 3, nc.sync)
                load_x_part(1, 0, nc.sync)
                load_x_part(1, 1, nc.scalar)
            nc.scalar.dma_start(out=wo_t[:], in_=woT[:])

            def xbd(j, d):
                return x_parts[(j, d // DG)][:, d % DG, :]

            # ---- rope (psum [128,512] f32 -> dst bf16) ----
            # Pool can't read PSUM: the four products (psum reads) go on DVE,
            # the two SBUF-only combine steps on Pool.
            def rope(ps, dst, j, csl=slice(0, 512)):
                a, b = ps[0:64, csl], ps[64:128, csl]
                cc = cos_b[j][:, csl]
                ss = sin_b[j][:, csl]
                n = csl.stop - csl.start
                t1 = sp.tile([64, 512], BF, tag="rt1", name="rt1")[:, 0:n]
                t2 = sp.tile([64, 512], BF, tag="rt2", name="rt2")[:, 0:n]
                nc.vector.tensor_mul(t1, a, cc)
                nc.vector.tensor_mul(t2, b, ss)
                nc.gpsimd.tensor_sub(dst[0:64, csl], t1, t2)
                t3 = sp.tile([64, 512], BF, tag="rt3", name="rt3")[:, 0:n]
                t4 = sp.tile([64, 512], BF, tag="rt4", name="rt4")[:, 0:n]
                nc.vector.tensor_mul(t3, a, ss)
                nc.vector.tensor_mul(t4, b, cc)
                nc.gpsimd.tensor_add(dst[64:128, csl], t3, t4)

            qTb = {}  # j -> [128, HQ, 512]

            def v_evac(ps, j, csl=slice(0, 512)):
                vt = sp.tile([128, 512], BF, tag="vt", name="vt", bufs=2)
                nc.vector.tensor_copy(vt[:, csl], ps[:, csl])
                for tt in range(csl.start // 128, csl.stop // 128):
                    c = j * 4 + tt
                    tp = psT.tile([128, 128], BF, tag="trans", name="trans")
                    nc.tensor.transpose(
                        tp[:], vt[:, tt * 128 : (tt + 1) * 128], ident[:]
                    )
                    nc.vector.tensor_copy(vaug[:, c, 0:128], tp[:])

            # ---- projection emission -------------------------------------
            # A "group" is one psum accumulation over all 32 d-chunks for one
            # projection head (K, V, or Q h). Emitted as 4 units of 8 matmuls
            # so attention/exp work can interleave between units.
            def proj_group_units(j, kind, h=None, csl=slice(0, 512)):
                """Returns a list of unit callables; shared psum tile."""
                box = {}
                ncols = csl.stop - csl.start

                def unit(u):
                    def run():
                        if u == 0:
                            box["ps"] = psP.tile(
                                [128, 512], F32, tag="proj", name="proj"
                            )
                        ps = box["ps"]
                        if kind == "q":
                            w = wq_t[h]
                        elif kind == "k":
                            w = wk_t
                        else:
                            w = wv_t
                        for d in range(u * 8, u * 8 + 8):
                            nc.tensor.matmul(
                                ps[:, csl],
                                w[:, d, :],
                                xbd(j, d)[:, csl],
                                start=(d == 0),
                                stop=(d == ND - 1),
                            )
                        if u == 3:
                            if kind == "q":
                                rope(ps, qTb[j][:, h, :], j, csl)
                            elif kind == "k":
                                rope(ps, kT[:, j * 512 : (j + 1) * 512], j, csl)
                            else:
                                v_evac(ps, j, csl)

                    return run

                return [unit(u) for u in range(4)]

            def proj_units(j):
                """All projection units for block j (K, V, Q0..Q3)."""
                qTb[j] = blk_tile([128, HQ, 512], BF, "qTb")
                units = []
                units += proj_group_units(j, "k")
                units += proj_group_units(j, "v")
                for h in range(HQ):
                    units += proj_group_units(j, "q", h=h)
                return units

            def proj_units_block0():
                """Block 0 split: K/V/Q0 (in 256-col halves so compute can
                start before the x parts fully land) run inline; Q1-Q3 are
                returned as filler for attn(0) — they only gate scores h1-h3,
                so their wq/x DMA moves off the proj(0) critical path."""
                qTb[0] = blk_tile([128, HQ, 512], BF, "qTb")
                inline, deferred = [], []
                ha, hb = slice(0, 256), slice(256, 512)
                inline += proj_group_units(0, "k", csl=ha)
                inline += proj_group_units(0, "v", csl=ha)
                inline += proj_group_units(0, "q", h=0, csl=ha)
                inline += proj_group_units(0, "k", csl=hb)
                inline += proj_group_units(0, "v", csl=hb)
                inline += proj_group_units(0, "q", h=0, csl=hb)
                for h in range(1, HQ):
                    for csl in (ha, hb):
                        deferred += proj_group_units(0, "q", h=h, csl=csl)
                return inline, deferred

            # ---- output projection units (block jb) ----------------------
            attnTb = {}  # j -> [128, HQ, 512]
            stage = {}

            def d_units(jb):
                # Last block: per-n stores on the (idle by then) sync hw
                # queue so the final store's wire time overlaps D(NB-1)
                # compute instead of draining after the last matmul.
                last = jb == NB - 1
                units = []
                for tt in range(4):
                    for n in range(DIM // 512):
                        def run(tt=tt, n=n):
                            at = attnTb[jb]
                            if n == 0:
                                stage[jb] = op.tile(
                                    [128, DIM], BF, tag="stg", name="stg"
                                )
                            ps = psP.tile([128, 512], F32, tag="proj", name="proj")
                            for h in range(HQ):
                                nc.tensor.matmul(
                                    ps[:],
                                    at[:, h, tt * 128 : (tt + 1) * 128],
                                    wo_t[:, h, n * 512 : (n + 1) * 512],
                                    start=(h == 0),
                                    stop=(h == HQ - 1),
                                )
                            nc.vector.tensor_copy(
                                stage[jb][:, n * 512 : (n + 1) * 512], ps[:]
                            )
                            t = jb * 4 + tt
                            if last:
                                nsl = slice(n * 512, (n + 1) * 512)
                                nc.sync.dma_start(
                                    out=outD[t * 128 : (t + 1) * 128, nsl],
                                    in_=stage[jb][:, nsl],
                                )
                            elif n == DIM // 512 - 1:
                                nc.gpsimd.dma_start(
                                    out=outD[t * 128 : (t + 1) * 128, :],
                                    in_=stage[jb][:],
                                )

                        units.append(run)
                return units

            # ---- attention for block j with filler interleaving ----------
            def attn_block(j, filler, pre_pops=0):
                """filler: deque of (is_proj, fn). Returns leftover D units
                (proj units must drain inside their block: the next block's
                scores depend on them and sit earlier in PE program order)."""
                cmax = 4 * j + 4
                jsl = slice(j * 512, (j + 1) * 512)
                attnTb[j] = blk_tile([128, HQ, 512], BF, "attnTb")
                mask_alt = [nc.vector, nc.gpsimd]
                popped = 0
                target = len(filler)
                for h in range(HQ):
                    heads_left = HQ - h
                    quota = popped + (target - popped + heads_left - 1) // heads_left
                    # block 0: this head's deferred Q group must run before
                    # its scores
                    for _ in range(pre_pops):
                        if filler:
                            filler.popleft()[1]()
                            popped += 1
                    et = {}
                    for c in range(cmax):
                        ps = psS.tile([128, 512], F32, tag="scores", name="scores")
                        nc.tensor.matmul(
                            ps[:],
                            kT[:, c * 128 : (c + 1) * 128],
                            qTb[j][:, h, :],
                            start=True,
                            stop=True,
                        )
                        e = ep.tile([128, 512], BF, tag="et", name="et")
                        nc.scalar.activation(
                            e[:], ps[:], mybir.ActivationFunctionType.Exp, scale=SC
                        )
                        if c // 4 == j:
                            mask_alt[c % 2].tensor_mul(
                                e[:], e[:], dmask[:, c % 4, :]
                            )
                        et[c] = e

                        def av_tile(tt):
                            # AV tile tt gates on chunks <= 4j+tt only: run
                            # it right after its diagonal chunk so its
                            # matmuls cover the tail chunks' exp latency
                            t = 4 * j + tt
                            po = psO.tile([128, 129], F32, tag="oaug", name="oaug")
                            for i, cc in enumerate(range(t + 1)):
                                nc.tensor.matmul(
                                    po[:],
                                    et[cc][:, tt * 128 : (tt + 1) * 128],
                                    vaug[:, cc, :],
                                    start=(i == 0),
                                    stop=(i == t),
                                )
                            rs = sp.tile([128, 1], F32, tag="rs", name="rs", bufs=2)
                            nc.vector.reciprocal(rs[:], po[:, 128:129])
                            an = sp.tile([128, 128], BF, tag="an", name="an", bufs=2)
                            # ACT-side scale keeps the transpose chain off
                            # the busy DVE queue
                            nc.scalar.mul(an[:], po[:, 0:128], rs[:])
                            tp = psT.tile([128, 128], BF, tag="trans", name="trans")
                            nc.tensor.transpose(tp[:], an[:], ident[:])
                            nc.vector.tensor_copy(
                                attnTb[j][:, h, tt * 128 : (tt + 1) * 128], tp[:]
                            )

                        # interleave filler to keep the PE busy while ACT exps
                        want = quota * (c + 1) // cmax
                        while filler and popped < want:
                            filler.popleft()[1]()
                            popped += 1
                    while filler and popped < quota:
                        filler.popleft()[1]()
                        popped += 1
                    for tt in range(4):
                        av_tile(tt)
                while filler:
                    filler.popleft()[1]()
                return deque()

            # ---- main schedule -------------------------------------------
            inline0, deferred0 = proj_units_block0()
            for f in inline0:
                f()
            carry = deque()
            for j in range(NB):
                # DMA for block j+2 goes out at the start of attn(j)
                if j + 2 < NB:
                    load_cossin(j + 2, nc.sync)
                    alloc_x(j + 2)
                    for g in range(4):
                        load_x_part(j + 2, g, nc.sync)
                filler = deque(carry)
                if j == 0:
                    filler.extend((True, f) for f in deferred0)
                if j + 1 < NB:
                    filler.extend((True, f) for f in proj_units(j + 1))
                if j >= 1:
                    filler.extend((False, f) for f in d_units(j - 1))
                carry = attn_block(j, filler)
            for f in carry:
                f[1]()
            for f in d_units(NB - 1):
                f()

    nc.finalize()
    return nc


def _build_v1(S, mask_mode):
    """v2 phase-serial kernel, kept for the non-causal fallback paths."""
    import concourse.mybir as mybir
    import concourse.tile as tile
    from concourse import bacc

    dt = mybir.dt
    BF, F32 = dt.bfloat16, dt.float32
    NB = S // 512
    NT = S // 128
    ND = DIM // 128
    SC = 1.0 / math.sqrt(HD)

    nc = bacc.Bacc(None, target_bir_lowering=False)
    xT = nc.declare_dram_parameter("xT", [128, ND, S], BF, isOutput=False)
    wqT = nc.declare_dram_parameter("wqT", [HQ, 128, ND, 128], BF, isOutput=False)
    wkT = nc.declare_dram_parameter("wkT", [128, ND, 128], BF, isOutput=False)
    wvT = nc.declare_dram_parameter("wvT", [128, ND, 128], BF, isOutput=False)
    woT = nc.declare_dram_parameter("woT", [128, HQ, DIM], BF, isOutput=False)
    cosT = nc.declare_dram_parameter("cosT", [64, S], F32, isOutput=False)
    sinT = nc.declare_dram_parameter("sinT", [64, S], F32, isOutput=False)
    identD = nc.declare_dram_parameter("identD", [128, 128], BF, isOutput=False)
    if mask_mode == "full":
        maskTD = nc.declare_dram_parameter("maskTD", [NT, 128, S], F32, isOutput=False)
    outD = nc.declare_dram_parameter("out", [S, DIM], F32, isOutput=True)

    with tile.TileContext(nc) as tc:
        with (
            tc.tile_pool(name="persist", bufs=1) as pp,
            tc.tile_pool(name="xb", bufs=8) as xp,
            tc.tile_pool(name="wstream", bufs=2) as wp,
            tc.tile_pool(name="et", bufs=NT + 1) as ep,
            tc.tile_pool(name="small", bufs=2) as sp,
            tc.tile_pool(name="oev", bufs=5) as op,
            tc.tile_pool(name="ps_proj", bufs=2, space="PSUM") as psP,
            tc.tile_pool(name="ps_scores", bufs=3, space="PSUM") as psS,
            tc.tile_pool(name="ps_oaug", bufs=2, space="PSUM") as psO,
            tc.tile_pool(name="ps_trans", bufs=1, space="PSUM") as psT,
        ):
            cos = pp.tile([64, S], F32, tag="cos", name="cos")
            sin = pp.tile([64, S], F32, tag="sin", name="sin")
            nc.scalar.dma_start(out=cos[:], in_=cosT[:])
            nc.scalar.dma_start(out=sin[:], in_=sinT[:])
            ident = pp.tile([128, 128], BF, tag="ident", name="ident")
            nc.scalar.dma_start(out=ident[:], in_=identD[:])
            wk_t = wp.tile([128, ND, 128], BF, tag="wk", name="wk", bufs=1)
            wv_t = wp.tile([128, ND, 128], BF, tag="wv", name="wv", bufs=1)
            qT = [pp.tile([128, S], BF, tag=f"qT{h}", name=f"qT{h}") for h in range(HQ)]
            kT = pp.tile([128, S], BF, tag="kT", name="kT")
            attnT = [pp.tile([128, S], BF, tag=f"attnT{h}", name=f"attnT{h}") for h in range(HQ)]
            vaug = pp.tile([128, NT, 129], BF, tag="vaug", name="vaug")

            def rope(ps, dst, bsl):
                a, b = ps[0:64, :], ps[64:128, :]
                cc, ss = cos[:, bsl], sin[:, bsl]
                t1 = sp.tile([64, 512], F32, tag="rt1", name="rt1")
                t2 = sp.tile([64, 512], F32, tag="rt2", name="rt2")
                nc.vector.tensor_mul(t1[:], a, cc)
                nc.vector.tensor_mul(t2[:], b, ss)
                nc.vector.tensor_sub(dst[0:64, bsl], t1[:], t2[:])
                t3 = sp.tile([64, 512], F32, tag="rt1", name="rt1")
                t4 = sp.tile([64, 512], F32, tag="rt2", name="rt2")
                nc.vector.tensor_mul(t3[:], a, ss)
                nc.vector.tensor_mul(t4[:], b, cc)
                nc.vector.tensor_add(dst[64:128, bsl], t3[:], t4[:])

            DG = ND // 4
            for b in range(NB):
                bsl = slice(b * 512, (b + 1) * 512)
                wq_first = wp.tile([128, ND, 128], BF, tag="wqh", name="wqh")
                nc.scalar.dma_start(out=wq_first[:], in_=wqT[0])
                xbp = []
                for g in range(4):
                    xg = xp.tile([128, DG, 512], BF, tag="xb", name="xb")
                    nc.sync.dma_start(
                        out=xg[:], in_=xT[:, g * DG : (g + 1) * DG, bsl]
                    )
                    xbp.append(xg)
                if b == 0:
                    nc.sync.dma_start(out=wk_t[:], in_=wkT[:])
                    nc.sync.dma_start(out=wv_t[:], in_=wvT[:])

                def xbd(d):
                    return xbp[d // DG][:, d % DG, :]

                for h in range(HQ):
                    if h == 0:
                        wq_t = wq_first
                    else:
                        wq_t = wp.tile([128, ND, 128], BF, tag="wqh", name="wqh")
                        nc.scalar.dma_start(out=wq_t[:], in_=wqT[h])
                    ps = psP.tile([128, 512], F32, tag="proj", name="proj")
                    for d in range(ND):
                        nc.tensor.matmul(
                            ps[:], wq_t[:, d, :], xbd(d),
                            start=(d == 0), stop=(d == ND - 1),
                        )
                    rope(ps, qT[h], bsl)
                ps = psP.tile([128, 512], F32, tag="proj", name="proj")
                for d in range(ND):
                    nc.tensor.matmul(
                        ps[:], wk_t[:, d, :], xbd(d),
                        start=(d == 0), stop=(d == ND - 1),
                    )
                rope(ps, kT, bsl)
                ps = psP.tile([128, 512], F32, tag="proj", name="proj")
                for d in range(ND):
                    nc.tensor.matmul(
                        ps[:], wv_t[:, d, :], xbd(d),
                        start=(d == 0), stop=(d == ND - 1),
                    )
                vt = sp.tile([128, 512], BF, tag="vt", name="vt")
                nc.vector.tensor_copy(vt[:], ps[:])
                for tt in range(4):
                    c = b * 4 + tt
                    tp = psT.tile([128, 128], BF, tag="trans", name="trans")
                    nc.tensor.transpose(tp[:], vt[:, tt * 128 : (tt + 1) * 128], ident[:])
                    nc.vector.tensor_copy(vaug[:, c, 0:128], tp[:])
                    nc.vector.memset(vaug[:, c, 128:129], 1.0)

            for h in range(HQ):
                for j in range(NB):
                    jsl = slice(j * 512, (j + 1) * 512)
                    cmax = NT
                    et = {}
                    for c in range(cmax):
                        ps = psS.tile([128, 512], F32, tag="scores", name="scores")
                        nc.tensor.matmul(
                            ps[:],
                            kT[:, c * 128 : (c + 1) * 128],
                            qT[h][:, jsl],
                            start=True,
                            stop=True,
                        )
                        if mask_mode == "full":
                            mt = sp.tile([128, 512], F32, tag="mt", name="mt")
                            nc.sync.dma_start(out=mt[:], in_=maskTD[c, :, jsl])
                            nc.vector.scalar_tensor_tensor(
                                ps[:], ps[:], SC, mt[:],
                                op0=mybir.AluOpType.mult, op1=mybir.AluOpType.add,
                            )
                            sc_exp = 1.0
                        else:
                            sc_exp = SC
                        e = ep.tile([128, 512], BF, tag="et", name="et")
                        nc.scalar.activation(
                            e[:], ps[:], mybir.ActivationFunctionType.Exp, scale=sc_exp
                        )
                        et[c] = e
                    for tt in range(4):
                        t = 4 * j + tt
                        cs = list(range(cmax))
                        po = psO.tile([128, 129], F32, tag="oaug", name="oaug")
                        for i, c in enumerate(cs):
                            nc.tensor.matmul(
                                po[:],
                                et[c][:, tt * 128 : (tt + 1) * 128],
                                vaug[:, c, :],
                                start=(i == 0),
                                stop=(i == len(cs) - 1),
                            )
                        rs = sp.tile([128, 1], F32, tag="rs", name="rs")
                        nc.vector.reciprocal(rs[:], po[:, 128:129])
                        an = sp.tile([128, 128], BF, tag="an", name="an")
                        nc.vector.tensor_scalar_mul(an[:], po[:, 0:128], rs[:])
                        tp = psT.tile([128, 128], BF, tag="trans", name="trans")
                        nc.tensor.transpose(tp[:], an[:], ident[:])
                        nc.vector.tensor_copy(
                            attnT[h][:, t * 128 : (t + 1) * 128], tp[:]
                        )

            NDB = DIM // 512
            for n in range(NDB):
                wot = wp.tile([128, HQ, 512], BF, tag="wo", name="wo", bufs=3)
                nc.scalar.dma_start(out=wot[:], in_=woT[:, :, n * 512 : (n + 1) * 512])
                for t in range(NT):
                    ps = psP.tile([128, 512], F32, tag="proj", name="proj")
                    for h in range(HQ):
                        nc.tensor.matmul(
                            ps[:],
                            attnT[h][:, t * 128 : (t + 1) * 128],
                            wot[:, h, :],
                            start=(h == 0),
                            stop=(h == HQ - 1),
                        )
                    ov = op.tile([128, 512], F32, tag="ov", name="ov")
                    if t % 2 == 0:
                        nc.scalar.copy(ov[:], ps[:])
                    else:
                        nc.vector.tensor_copy(ov[:], ps[:])
                    st_eng = nc.sync if t % 2 == 0 else nc.scalar
                    st_eng.dma_start(
                        out=outD[t * 128 : (t + 1) * 128, n * 512 : (n + 1) * 512],
                        in_=ov[:],
                    )

    nc.finalize()
    return nc


def _prep_inputs(x, wq, wk, wv, wo, freqs_cos, freqs_sin, mask, S, mask_mode):
    """Host-side shard + layout prep. Returns list of in_maps (one per core)."""
    bf = ml_dtypes.bfloat16
    ND = DIM // 128
    NT = S // 128
    x2 = np.ascontiguousarray(x.reshape(S, DIM))
    if mask_mode == "causal":
        # block-major [128, NB, ND, 512]: xT[p, b, d, s'] = x[b*512+s', 128*d+p]
        NBb = S // 512
        xT = np.ascontiguousarray(
            x2.T.reshape(ND, 128, NBb, 512).transpose(1, 2, 0, 3)
        ).astype(bf)
        # block 0 half-major: x0T[p, half, d, s'] = x[half*256+s', 128*d+p]
        x0T = np.ascontiguousarray(
            x2[0:512].T.reshape(ND, 128, 2, 256).transpose(1, 2, 0, 3)
        ).astype(bf)
    else:
        # [128, ND, S] partition-major: xT[p, d, s] = x[s, 128*d + p]
        xT = np.ascontiguousarray(
            x2.T.reshape(ND, 128, S).transpose(1, 0, 2)
        ).astype(bf)
    cosT = np.ascontiguousarray(freqs_cos.T).astype(np.float32)
    sinT = np.ascontiguousarray(freqs_sin.T).astype(np.float32)
    ident = np.eye(128, dtype=bf)
    if mask_mode == "causal":
        r = np.arange(128)[:, None]
        col = np.arange(512)[None, :]
        dmask = np.stack(
            [(128 * p + r <= col) for p in range(4)], axis=1
        ).astype(bf)  # [128, 4, 512]
    elif mask_mode == "full":
        maskT = np.ascontiguousarray(mask.T).astype(np.float32).reshape(NT, 128, S)

    in_maps = []
    for m in range(NCORES):
        wq_l = wq[m * 512 : (m + 1) * 512]  # [512, 4096]
        wq_l = wq_l.reshape(HQ, HD, DIM)[:, _HEAD_PERM, :].reshape(512, DIM)
        wqT_l = np.ascontiguousarray(
            wq_l.T.reshape(ND, 128, HQ, 128).transpose(2, 1, 0, 3)
        ).astype(bf)
        wk_l = wk[m * 128 : (m + 1) * 128][_HEAD_PERM]
        wkT_l = np.ascontiguousarray(
            wk_l.T.reshape(ND, 128, 128).transpose(1, 0, 2)
        ).astype(bf)
        wv_l = wv[m * 128 : (m + 1) * 128]
        wvT_l = np.ascontiguousarray(
            wv_l.T.reshape(ND, 128, 128).transpose(1, 0, 2)
        ).astype(bf)
        woT_l = np.ascontiguousarray(
            wo[:, m * 512 : (m + 1) * 512].T.reshape(HQ, 128, DIM).transpose(1, 0, 2)
        ).astype(bf)
        im = {
            "xT": xT,
            "wqT": wqT_l,
            "wkT": wkT_l,
            "wvT": wvT_l,
            "woT": woT_l,
            "cosT": cosT,
            "sinT": sinT,
            "identD": ident,
        }
        if mask_mode == "causal":
            im["dmaskD"] = dmask
            im["x0T"] = x0T
        elif mask_mode == "full":
            im["maskTD"] = maskT
        in_maps.append(im)
    return in_maps


def _detect_mask_mode(mask):
    if not np.any(mask):
        return "none"
    S = mask.shape[0]
    causal = np.where(np.triu(np.ones((S, S), dtype=bool), k=1), -1e9, 0.0).astype(
        np.float32
    )
    if np.array_equal(mask, causal):
        return "causal"
    return "full"


def kernel(x, wq, wk, wv, wo, freqs_cos, freqs_sin, cache_k, cache_v, mask, start_pos):
    """Full inputs in, full output out. start_pos/caches are no-ops for these
    shapes (the reference's dynamic_update_slice clamps to a full overwrite)."""
    global LAST_EXEC_NS, LAST_RESULT
    from concourse.bass_utils import run_bass_kernel_spmd

    x = np.asarray(x, dtype=np.float32)
    B, S, _ = x.shape
    assert B == 1
    mask = np.asarray(mask, dtype=np.float32)
    mode = _detect_mask_mode(mask)
    if mode == "causal":
        nc = _build_causal(S)
    else:
        nc = _build_v1(S, mode)
    in_maps = _prep_inputs(
        x, np.asarray(wq, np.float32), np.asarray(wk, np.float32),
        np.asarray(wv, np.float32), np.asarray(wo, np.float32),
        np.asarray(freqs_cos, np.float32), np.asarray(freqs_sin, np.float32),
        mask, S, mode,
    )
    import os

    tmpdir = os.environ.get("BASS_KERNEL_TMPDIR") or None
    if tmpdir:
        os.makedirs(tmpdir, exist_ok=True)
    res = run_bass_kernel_spmd(nc, in_maps, list(range(NCORES)), tmpdir=tmpdir)
    LAST_EXEC_NS = res.exec_time_ns
    LAST_RESULT = res
    acc = np.zeros((S, DIM), dtype=np.float64)
    for r in res.results:
        acc += r["out"].astype(np.float64)
    return acc.astype(np.float32).reshape(1, S, DIM)
